# revision 33
# baseline (speedup 1.0000x reference)
"""Trainium2 Bass kernel for KernelPooling (count-sketch polynomial pooling).

One image per NeuronCore (B=8 = n_cores). Per core:
  fft(count_sketch_t(x[n]))[k] = sum_c A_t[k,c] x[n,c] with
  A_t[k,c] = s_t(c)*exp(-2pi i k h_t(c)/D)  -> fp8 DoubleRow matmuls (PE,
  contraction 512 = 2 passes of 256)
  cp1 = xf0*xf1, cp2 = cp1*xf2 elementwise (DVE bf16; 1/32 pre-scale
  applied at PSUM evacuation, undone in the IFFT constants)
  m_t[k] = sum_n cp_t[n,k] via bf16 ones-matmuls (fp32 PSUM accum)
  xi_t = irfft(m_t) via radix-64 Cooley-Tukey as tiny fp32 matmuls
  phi = l2norm(signed_sqrt([a0, a1*mean(x), a2*xi1, a3*xi2]))  all on device
"""
import sys
sys.path.insert(0, "/opt/trn_rl_repo")
from contextlib import ExitStack

import numpy as np
import ml_dtypes

from concourse import bass, tile, bacc, mybir
from concourse.bass_utils import run_bass_kernel_spmd

BF16 = mybir.dt.bfloat16
F32 = mybir.dt.float32
FP8 = mybir.dt.float8e4
AF = mybir.ActivationFunctionType
ALU = mybir.AluOpType
AX = mybir.AxisListType
PSUM = bass.MemorySpace.PSUM
PM = mybir.MatmulPerfMode

D = 4096
C = 512
B = 8
N = 784            # 28*28 positions per image
KF = 2049          # rfft bins
NT, NSZ = 7, 112   # position tiles
KW = 512           # max chunk width (bin 2048 handled separately)
CHUNKS = [(0, 512), (512, 512), (1024, 512), (1536, 512)]
NCH = len(CHUNKS)
SCALE = 1.0 / 32   # xf pre-scale so cp products fit fp8 range
EPS = 1e-12
NPHI = 1 + C + 2 * D  # 8705

_cache = {}


def _build_program(a0, a1):
    """Build the bass program. a0, a1 (floats) get baked in; array consts are inputs."""
    nc = bacc.Bacc("TRN2", target_bir_lowering=False, debug=False, num_devices=B)

    xb_d = nc.dram_tensor("xb", [N, C], BF16, kind="ExternalInput").ap()
    xp_d = nc.dram_tensor("xp8", [2, 128, 2, N], FP8, kind="ExternalInput").ap()
    A_d = nc.dram_tensor("Abig", [6, 2, 128, 2, 2048], FP8, kind="ExternalInput").ap()
    AN_d = nc.dram_tensor("Anyq", [128, 2, 2, 16], FP8, kind="ExternalInput").ap()
    W_d = nc.dram_tensor("Wc", [3, 32, 64], F32, kind="ExternalInput").ap()   # WR,WI,WnI
    CW_d = nc.dram_tensor("Cw", [4, 64, 64], F32, kind="ExternalInput").ap()  # CR1,CI1,CR2,CI2
    G_d = nc.dram_tensor("Gc", [2, 64, 64], F32, kind="ExternalInput").ap()   # GcosT,GnegsinT
    UV_d = nc.dram_tensor("uv", [4, 64], F32, kind="ExternalInput").ap()      # u1,v1,u2,v2

    mrow_d = [nc.dram_tensor(f"mrow{qi}", [1, KF], F32, kind="Internal").ap()
              for qi in range(4)]  # m1R,m1I,m2R,m2I
    phi0_d = nc.dram_tensor("phi0", [1, 1], F32, kind="ExternalOutput").ap()
    pfirst_d = nc.dram_tensor("pfirst", [C, 1], F32, kind="ExternalOutput").ap()
    pxi_d = [nc.dram_tensor(f"pxi{t}", [64, 64], F32, kind="ExternalOutput").ap()
             for t in (1, 2)]

    zsigned = float(np.sign(a0) * np.sqrt(abs(a0) + EPS))
    c0 = float(abs(a0) + NPHI * EPS)
    s1scale = float(a1 / N)
    s1sign = 1.0 if a1 >= 0 else -1.0

    with tile.TileContext(nc) as tc, ExitStack() as ctx:
        consts = ctx.enter_context(tc.tile_pool(name="consts", bufs=1))
        apool = ctx.enter_context(tc.tile_pool(name="apool", bufs=2))
        xpool = ctx.enter_context(tc.tile_pool(name="xp", bufs=1))
        stpool = ctx.enter_context(tc.tile_pool(name="stage", bufs=2))
        sfin = ctx.enter_context(tc.tile_pool(name="sfin", bufs=1))
        fin = ctx.enter_context(tc.tile_pool(name="fin", bufs=1))

        # ---- critical-path loads first: xp8 gates the first matmuls ----
        XP = []
        for pair in range(2):
            t = xpool.tile([128, 2, N], FP8, name=f"xp{pair}", tag=f"xp{pair}")
            nc.sync.dma_start(t[:, 0], xp_d[pair, :, 0])
            nc.scalar.dma_start(t[:, 1], xp_d[pair, :, 1])
            XP.append(t)
        AN_sb = xpool.tile([128, 2, 2, 16], FP8, name="anyq", tag="anyq")
        nc.sync.dma_start(AN_sb[:], AN_d[:])

        def load_a_tiles(ci, first):
            klo, kw = CHUNKS[ci]
            row = []
            for q in range(6):
                rr = []
                for pair in range(2):
                    t = apool.tile([128, 2, KW], FP8, name=f"a{q}{pair}",
                                   tag=f"a{q}{pair}")
                    if first:
                        eng = (nc.scalar, nc.sync, nc.gpsimd)[(2 * q + pair) % 3]
                    else:
                        eng = nc.gpsimd if (q + pair) % 2 == 0 else nc.sync
                    eng.dma_start(t[:, :, :kw], A_d[q, pair, :, :, klo:klo + kw])
                    rr.append(t)
                row.append(rr)
            return row

        a_pref = load_a_tiles(0, True)
        ones_bf = consts.tile([NSZ, 1], BF16, name="onesbf", tag="onesbf")
        nc.vector.memset(ones_bf[:], 1.0)

        # ---- non-critical loads, issued on idle queues at startup ----
        xb_sb = []
        for nt in range(NT):
            t = xpool.tile([NSZ, C], BF16, name=f"xb{nt}", tag=f"xb{nt}")
            nc.gpsimd.dma_start(t[:], xb_d[nt * NSZ:(nt + 1) * NSZ, :])
            xb_sb.append(t)
        ones1x64 = consts.tile([1, 64], F32, name="o1x64", tag="o1x64")
        nc.vector.memset(ones1x64[:], 1.0)
        ones1x128 = consts.tile([1, 128], F32, name="o1x128", tag="o1x128")
        nc.vector.memset(ones1x128[:], 1.0)
        onesP64 = consts.tile([64, 1], F32, name="oP64", tag="oP64")
        nc.vector.memset(onesP64[:], 1.0)
        onesP128 = consts.tile([128, 1], F32, name="oP128", tag="oP128")
        nc.vector.memset(onesP128[:], 1.0)
        eps128 = consts.tile([128, 1], F32, name="eps128", tag="eps128")
        nc.vector.memset(eps128[:], EPS)
        W_sb = []
        for i in range(3):
            t = consts.tile([32, 64], F32, name=f"W{i}", tag=f"W{i}")
            nc.gpsimd.dma_start(t[:], W_d[i])
            W_sb.append(t)
        CW_sb = []
        for i in range(4):
            t = consts.tile([64, 64], F32, name=f"CW{i}", tag=f"CW{i}")
            nc.gpsimd.dma_start(t[:], CW_d[i])
            CW_sb.append(t)
        G_sb = []
        for i in range(2):
            t = consts.tile([64, 64], F32, name=f"G{i}", tag=f"G{i}")
            nc.gpsimd.dma_start(t[:], G_d[i])
            G_sb.append(t)
        UV_sb = []
        for i in range(4):
            t = consts.tile([1, 64], F32, name=f"uv{i}", tag=f"uv{i}")
            nc.gpsimd.dma_start(t[:], UV_d[i:i + 1, :])
            UV_sb.append(t)

        # ---- Nyquist bin 2048: xfN[n, t] = sum_c x[n,c] s_t(c) (-1)^{h_t(c)}
        # (s_t pre-scaled by SCALE on host). Hidden under the chunk-0 A DMA.
        with tc.tile_pool(name="psN0", bufs=2, space=PSUM) as psN0, \
             tc.tile_pool(name="psN1", bufs=1, space=PSUM) as psN1, \
             tc.tile_pool(name="nyq", bufs=1) as nyq:
            cpn = nyq.tile([NSZ, 2 * NT], F32, name="cpn", tag="cpn")
            nyx = nyq.tile([NSZ, 3 * NT], F32, name="nyx", tag="nyx")
            for nt in range(NT):
                pn = psN0.tile([NSZ, 3], F32, name="pn", tag="pn")
                for pair in range(2):
                    nc.tensor.matmul(
                        pn[:], XP[pair][:, :, nt * NSZ:(nt + 1) * NSZ],
                        AN_sb[:, pair, :, 0:3],
                        start=(pair == 0), stop=(pair == 1),
                        perf_mode=PM.DoubleRow)
                xn = nyx[:, 3 * nt:3 * nt + 3]
                nc.vector.tensor_copy(xn, pn[:])
                # cp1N = xf0N*xf1N ; cp2N = cp1N*xf2N
                nc.vector.tensor_mul(cpn[:, 2 * nt:2 * nt + 1],
                                     xn[:, 0:1], xn[:, 1:2])
                nc.vector.tensor_mul(cpn[:, 2 * nt + 1:2 * nt + 2],
                                     cpn[:, 2 * nt:2 * nt + 1], xn[:, 2:3])
            ones32 = nyq.tile([NSZ, 1], F32, name="ones32", tag="ones32")
            nc.vector.memset(ones32[:], 1.0)
            pm = psN1.tile([1, 2 * NT], F32, name="pmn", tag="pmn")
            nc.tensor.matmul(pm[:], ones32[:], cpn[:], start=True, stop=True)
            mn = sfin.tile([1, 4], F32, name="mn", tag="mn")
            # m1R[2048], m2R[2048]; imag parts are zero
            nc.vector.tensor_reduce(
                mn[:, 0:1], pm[:].rearrange("p (a b) -> p a b", b=2)[:, :, 0],
                AX.X, ALU.add)
            nc.vector.tensor_reduce(
                mn[:, 2:3], pm[:].rearrange("p (a b) -> p a b", b=2)[:, :, 1],
                AX.X, ALU.add)
            nc.vector.memset(mn[:, 1:2], 0.0)
            nc.vector.memset(mn[:, 3:4], 0.0)
            for qi in (0, 2):
                nc.gpsimd.dma_start(mrow_d[qi][:, 2048:2049], mn[:, qi:qi + 1])


        # ---- first-order term early: rides the startup DMA window ----
        # first = a1 * mean_n x (per channel)
        absf, sgnf = [], []
        with tc.tile_pool(name="psF", bufs=4, space=PSUM) as psF:
            for ct in range(4):
                fp = psF.tile([128, 1], F32, name="fp", tag="fp")
                for nt in range(NT):
                    nc.tensor.matmul(
                        fp[:], xb_sb[nt][:, ct * 128:(ct + 1) * 128],
                        ones_bf[:],
                        start=(nt == 0), stop=(nt == NT - 1))
                av = sfin.tile([128, 1], F32, name=f"absf{ct}", tag=f"absf{ct}")
                nc.scalar.activation(av[:], fp[:], AF.Abs, scale=s1scale)
                sv = sfin.tile([128, 1], F32, name=f"sgnf{ct}", tag=f"sgnf{ct}")
                nc.scalar.activation(sv[:], fp[:], AF.Sign, scale=s1sign)
                absf.append(av)
                sgnf.append(sv)

        with tc.tile_pool(name="xfpool", bufs=2) as xfpool, \
             tc.tile_pool(name="cppool", bufs=2) as cppool, \
             tc.tile_pool(name="tmppool", bufs=1) as tmppool, \
             tc.tile_pool(name="psA", bufs=3, space=PSUM) as psA, \
             tc.tile_pool(name="psM", bufs=2, space=PSUM) as psM:

            pending = []

            def emit_mred(job):
                cpt, klo, kw = job
                last = klo + kw >= 2048
                for qi in range(4):
                    ps = psM.tile([1, KW], F32, name="psm", tag="psm")
                    for j in range(NT):
                        nc.tensor.matmul(
                            ps[:, :kw], ones_bf[:],
                            cpt[qi][:, j, :kw],
                            start=(j == 0), stop=(j == NT - 1))
                    st = stpool.tile([1, KW], F32, name="mstage", tag="mstage")
                    if last:
                        nc.scalar.copy(st[:, :kw], ps[:, :kw])
                    else:
                        nc.vector.tensor_copy(st[:, :kw], ps[:, :kw])
                    nc.sync.dma_start(
                        mrow_d[qi][:, klo:klo + kw], st[:, :kw])

            def emit_products_half(xf, cp, tA, tB, lo, hi, kw):
                # stage B on position tiles [lo, hi): cp1=xf0*xf1, cp2=cp1*xf2
                sl = (slice(None), slice(lo, hi), slice(0, kw))
                R0, I0, R1, I1, R2, I2 = (t[sl] for t in xf)
                cp1R, cp1I, cp2R, cp2I = (t[sl] for t in cp)
                a, b = tA[sl], tB[sl]
                nc.vector.tensor_mul(a, R0, R1)
                nc.vector.tensor_mul(b, I0, I1)
                nc.vector.tensor_sub(cp1R, a, b)
                nc.vector.tensor_mul(a, R0, I1)
                nc.vector.tensor_mul(b, I0, R1)
                nc.vector.tensor_add(cp1I, a, b)
                nc.vector.tensor_mul(a, cp1R, R2)
                nc.vector.tensor_mul(b, cp1I, I2)
                nc.vector.tensor_sub(cp2R, a, b)
                nc.vector.tensor_mul(a, cp1R, I2)
                nc.vector.tensor_mul(b, cp1I, R2)
                nc.vector.tensor_add(cp2I, a, b)

            for ci in range(NCH):
                klo, kw = CHUNKS[ci]
                a_sb = a_pref if ci == 0 else load_a_tiles(ci, False)

                # stage A: xf_q[n, k] = SCALE * sum_c x[n,c] A_q[c,k]
                # q-pairs share one [NSZ, 2, KW] psum tile -> one evac copy
                xfg = [xfpool.tile([NSZ, NT, 2, KW], BF16, name=f"xfg{g}",
                                   tag=f"xfg{g}") for g in range(3)]
                xf = [xfg[q // 2][:, :, q % 2, :] for q in range(6)]
                cp = [cppool.tile([NSZ, NT, KW], BF16, name=f"cp{i}", tag=f"cp{i}")
                      for i in range(4)]
                tA = tmppool.tile([NSZ, NT, KW], BF16, name="tA", tag="tA")
                tB = tmppool.tile([NSZ, NT, KW], BF16, name="tB", tag="tB")
                for nt in range(NT):
                    for g in range(3):
                        ps = psA.tile([NSZ, 2, KW], F32, name="psa", tag="psa")
                        for qq in range(2):
                            for pair in range(2):
                                nc.tensor.matmul(
                                    ps[:, qq, :kw],
                                    XP[pair][:, :, nt * NSZ:(nt + 1) * NSZ],
                                    a_sb[2 * g + qq][pair][:, :, :kw],
                                    start=(pair == 0), stop=(pair == 1),
                                    perf_mode=PM.DoubleRow)
                        nc.scalar.activation(xfg[g][:, nt, :, :kw], ps[:, :, :kw],
                                             AF.Copy, scale=SCALE)
                    if nt == 4:
                        # first-half products overlap this chunk's stage A
                        emit_products_half(xf, cp, tA, tB, 0, 5, kw)
                emit_products_half(xf, cp, tA, tB, 5, NT, kw)

                pending.append((cp, klo, kw))
                if ci >= 1:
                    emit_mred(pending.pop(0))
            while pending:
                emit_mred(pending.pop(0))

        # ================= final phase =================
        with tc.tile_pool(name="psT", bufs=1, space=PSUM) as psT, \
             tc.tile_pool(name="psY", bufs=1, space=PSUM) as psY, \
             tc.tile_pool(name="psN", bufs=1, space=PSUM) as psN, \
             tc.tile_pool(name="psB", bufs=1, space=PSUM) as psB:

            y_ps = []
            s_t = []
            mmTs = []
            for t in range(2):
                mmT = []
                for q in range(2):  # R, I
                    mt = fin.tile([32, 64], F32, name=f"mmT{t}{q}", tag=f"mmT{t}{q}")
                    nc.sync.dma_start(
                        mt[:],
                        mrow_d[2 * t + q][:, 0:2048]
                        .rearrange("p (a b) -> (p a) b", a=32))
                    mmT.append(mt)
                mmTs.append(mmT)
            m0s = [mmTs[0][0][0:1, 0:1], mmTs[1][0][0:1, 0:1]]
            mNs = [mn[:, 0:1], mn[:, 2:3]]
            TRs, TIs, Tps, crows = [], [], [], []
            for t in range(2):  # stage 1 DFT for both orders first
                mmT = mmTs[t]
                TR = psT.tile([64, 64], F32, name="TR", tag=f"TR{t}")
                nc.tensor.matmul(TR[:], mmT[0][:], W_sb[0][:], start=True, stop=False)
                nc.tensor.matmul(TR[:], mmT[1][:], W_sb[2][:], start=False, stop=True)
                TI = psT.tile([64, 64], F32, name="TI", tag=f"TI{t}")
                nc.tensor.matmul(TI[:], mmT[0][:], W_sb[1][:], start=True, stop=False)
                nc.tensor.matmul(TI[:], mmT[1][:], W_sb[0][:], start=False, stop=True)
                TRs.append(TR)
                TIs.append(TI)
            for t in range(2):  # twiddle + correction row (DVE)
                TR, TI = TRs[t], TIs[t]
                CR, CI = CW_sb[2 * t], CW_sb[2 * t + 1]
                ta = fin.tile([64, 64], F32, name=f"ta{t}", tag=f"ta{t}")
                tb = fin.tile([64, 64], F32, name=f"tb{t}", tag=f"tb{t}")
                TpR = fin.tile([64, 64], F32, name=f"TpR{t}", tag=f"TpR{t}")
                TpI = fin.tile([64, 64], F32, name=f"TpI{t}", tag=f"TpI{t}")
                nc.vector.tensor_mul(ta[:], TR[:], CR[:])
                nc.vector.tensor_mul(tb[:], TI[:], CI[:])
                nc.vector.tensor_sub(TpR[:], ta[:], tb[:])
                nc.vector.tensor_mul(ta[:], TR[:], CI[:])
                nc.vector.tensor_mul(tb[:], TI[:], CR[:])
                nc.vector.tensor_add(TpI[:], ta[:], tb[:])
                crow = fin.tile([1, 64], F32, name=f"crow{t}", tag=f"crow{t}")
                tmpr = fin.tile([1, 64], F32, name=f"tmpr{t}", tag=f"tmpr{t}")
                nc.vector.tensor_scalar_mul(tmpr[:], UV_sb[2 * t + 1][:], mNs[t])
                nc.vector.scalar_tensor_tensor(
                    crow[:], UV_sb[2 * t][:], m0s[t], tmpr[:],
                    op0=ALU.mult, op1=ALU.add)
                Tps.append((TpR, TpI))
                crows.append(crow)
            for t in range(2):  # stage 2 + correction broadcast
                TpR, TpI = Tps[t]
                y = psY.tile([64, 64], F32, name=f"y{t}", tag=f"y{t}")
                nc.tensor.matmul(y[:], G_sb[0][:], TpR[:], start=True, stop=False)
                nc.tensor.matmul(y[:], G_sb[1][:], TpI[:], start=False, stop=False)
                nc.tensor.matmul(y[:], ones1x64[:], crows[t][:], start=False,
                                 stop=True, skip_group_check=True)
                y_ps.append(y)
                st = fin.tile([64, 1], F32, name=f"st{t}", tag=f"st{t}")
                nc.vector.tensor_reduce(st[:], y[:], AX.X, ALU.add,
                                        apply_absolute_value=True)
                s_t.append(st)

            # norm total = sum|y1| + sum|y2| + sum|first| + (|a0| + NPHI*eps)
            tot = psN.tile([1, 1], F32, name="tot", tag="tot")
            nc.tensor.matmul(tot[:], onesP64[:], s_t[0][:], start=True, stop=False,
                             skip_group_check=True)
            nc.tensor.matmul(tot[:], onesP64[:], s_t[1][:], start=False, stop=False,
                             skip_group_check=True)
            for ct in range(4):
                nc.tensor.matmul(tot[:], onesP128[:], absf[ct][:],
                                 start=False, stop=(ct == 3),
                                 skip_group_check=True)
            tot_sb = fin.tile([1, 1], F32, name="tot_sb", tag="tot_sb")
            nc.scalar.activation(tot_sb[:], tot[:], AF.Copy, bias=c0)
            rec = fin.tile([1, 1], F32, name="rec", tag="rec")
            nc.vector.reciprocal(rec[:], tot_sb[:])
            ninv = fin.tile([1, 1], F32, name="ninv", tag="ninv")
            nc.scalar.sqrt(ninv[:], rec[:])
            nv128_ps = psB.tile([128, 1], F32, name="nv128", tag="nv128")
            nc.tensor.matmul(nv128_ps[:], ones1x128[:], ninv[:], start=True, stop=True)
            nv128 = fin.tile([128, 1], F32, name="nv128sb", tag="nv128sb")
            nc.scalar.copy(nv128[:], nv128_ps[:])
            nv64 = nv128[0:64]

            # phi pieces
            ph0 = fin.tile([1, 1], F32, name="ph0", tag="ph0")
            nc.vector.tensor_scalar_mul(ph0[:], ninv[:], zsigned)
            nc.sync.dma_start(phi0_d[:], ph0[:])
            for ct in range(4):
                sqf = fin.tile([128, 1], F32, name=f"sqf{ct}", tag=f"sqf{ct}")
                nc.scalar.activation(sqf[:], absf[ct][:], AF.Sqrt, bias=eps128[:])
                pmf = fin.tile([128, 1], F32, name=f"pmf{ct}", tag=f"pmf{ct}")
                nc.vector.tensor_mul(pmf[:], sqf[:], sgnf[ct][:])
                phf = fin.tile([128, 1], F32, name=f"phf{ct}", tag=f"phf{ct}")
                nc.vector.tensor_scalar_mul(phf[:], pmf[:], nv128[:])
                nc.sync.dma_start(pfirst_d[ct * 128:(ct + 1) * 128, :], phf[:])
            for t in range(2):
                ab = fin.tile([64, 64], F32, name=f"ab{t}", tag=f"ab{t}")
                nc.scalar.activation(ab[:], y_ps[t][:], AF.Abs)
                sq = fin.tile([64, 64], F32, name=f"sq{t}", tag=f"sq{t}")
                nc.scalar.activation(sq[:], ab[:], AF.Sqrt, bias=eps128[:64])
                sg = fin.tile([64, 64], F32, name=f"sg{t}", tag=f"sg{t}")
                nc.scalar.activation(sg[:], y_ps[t][:], AF.Sign)
                pm = fin.tile([64, 64], F32, name=f"pm{t}", tag=f"pm{t}")
                nc.vector.tensor_mul(pm[:], sq[:], sg[:])
                phx = fin.tile([64, 64], F32, name=f"phx{t}", tag=f"phx{t}")
                nc.vector.tensor_scalar_mul(phx[:], pm[:], nv64[:])
                nc.sync.dma_start(pxi_d[t][:], phx[:])

    nc.compile()
    return nc


def _host_prep(x, alpha, h_idx, s_bits):
    """Per-core input maps: fp8 image/DFT layouts + fp32 IFFT constants."""
    x = np.asarray(x, np.float32)
    alpha = np.asarray(alpha, np.float64)
    h_idx = np.asarray(h_idx).astype(np.int64)
    s_bits = np.asarray(s_bits).astype(np.int64)
    signs = (2 * s_bits - 1).astype(np.float64)

    # DFT matrices A_t[c, k] (fp8), packed for DoubleRow:
    # A8[q, pair, p, i, k] = A_q[pair*256 + i*128 + p, k]
    k = np.arange(2048, dtype=np.float64)[:, None]
    Abig = np.empty((6, C, 2048), ml_dtypes.float8_e4m3)
    for t in range(3):
        ang = -2.0 * np.pi * ((k * h_idx[t][None, :]) % D) / D
        Abig[2 * t] = (np.cos(ang) * signs[t][None, :]).T.astype(
            ml_dtypes.float8_e4m3)
        Abig[2 * t + 1] = (np.sin(ang) * signs[t][None, :]).T.astype(
            ml_dtypes.float8_e4m3)
    A8 = np.ascontiguousarray(
        Abig.reshape(6, 2, 2, 128, 2048).transpose(0, 1, 3, 2, 4))
    # AN8[p, pair, i, t] = SCALE * s_t(c) * (-1)^{h_t(c)},  c = pair*256+i*128+p
    Anyq = np.empty((C, 3), np.float64)
    for t in range(3):
        Anyq[:, t] = SCALE * signs[t] * ((-1.0) ** (h_idx[t] % 2))
    AN8 = np.zeros((128, 2, 2, 16), ml_dtypes.float8_e4m3)
    AN8[:, :, :, 0:3] = np.ascontiguousarray(
        Anyq.reshape(2, 2, 128, 3).transpose(2, 0, 1, 3)).astype(
        ml_dtypes.float8_e4m3)

    j0 = np.arange(64, dtype=np.float64)[None, :]
    k2 = np.arange(32, dtype=np.float64)[:, None]
    k1 = np.arange(64, dtype=np.float64)[:, None]
    Wc = np.empty((3, 32, 64), np.float32)
    Wc[0] = np.cos(2 * np.pi * k2 * j0 / 64)
    Wc[1] = np.sin(2 * np.pi * k2 * j0 / 64)
    Wc[2] = -Wc[1]
    Cw = np.empty((4, 64, 64), np.float32)
    uv = np.empty((4, 64), np.float32)
    for t in range(2):
        # undo the SCALE^(t+2) applied on-device to cp_{t+1}
        sig = 2.0 * alpha[2 + t] / (D * N) / SCALE ** (t + 2)
        Cw[2 * t] = sig * np.cos(2 * np.pi * k1 * j0 / D)
        Cw[2 * t + 1] = sig * np.sin(2 * np.pi * k1 * j0 / D)
        uv[2 * t] = -alpha[2 + t] / (D * N) / SCALE ** (t + 2)
        uv[2 * t + 1] = (alpha[2 + t] / (D * N) / SCALE ** (t + 2)
                         * ((-1.0) ** np.arange(64)))
    g = 2 * np.pi * k1 * np.arange(64)[None, :] / 64
    Gc = np.empty((2, 64, 64), np.float32)
    Gc[0] = np.cos(g)
    Gc[1] = -np.sin(g)

    in_maps = []
    xf = x.reshape(B, N, C)
    for b in range(B):
        # xp8[pair, p, i, n] = x[n, pair*256 + i*128 + p]
        xT = xf[b].T.reshape(2, 2, 128, N).transpose(0, 2, 1, 3)
        in_maps.append({
            "xb": xf[b].astype(ml_dtypes.bfloat16),
            "xp8": np.ascontiguousarray(xT).astype(ml_dtypes.float8_e4m3),
            "Abig": A8, "Anyq": AN8, "Wc": Wc, "Cw": Cw, "Gc": Gc, "uv": uv,
        })
    return in_maps, float(alpha[0]), float(alpha[1])


def kernel(x, alpha, h_idx, s_bits, _trace=False, _tmpdir=None):
    in_maps, a0, a1 = _host_prep(x, alpha, h_idx, s_bits)
    key = (round(a0, 12), round(a1, 12))
    if key not in _cache:
        _cache[key] = _build_program(a0, a1)
    nc = _cache[key]
    res = run_bass_kernel_spmd(nc, in_maps, core_ids=list(range(B)),
                               trace=_trace, tmpdir=_tmpdir)
    kernel.last_result = res
    out = np.empty((B, NPHI), np.float32)
    for b in range(B):
        r = res.results[b]
        out[b, 0] = r["phi0"][0, 0]
        out[b, 1:1 + C] = r["pfirst"].reshape(C)
        out[b, 1 + C:1 + C + D] = r["pxi1"].reshape(D)
        out[b, 1 + C + D:] = r["pxi2"].reshape(D)
    return out


# revision 34
# speedup vs baseline: 1.1766x; 1.1766x over previous
"""Trainium2 Bass kernel for KernelPooling (count-sketch polynomial pooling).

One image per NeuronCore (B=8 = n_cores). Per core:
  fft(count_sketch_t(x[n]))[k] = sum_c A_t[k,c] x[n,c] with
  A_t[k,c] = s_t(c)*exp(-2pi i k h_t(c)/D)  -> fp8 DoubleRow matmuls (PE,
  contraction 512 = 2 passes of 256)
  cp1 = xf0*xf1, cp2 = cp1*xf2 elementwise (DVE bf16; 1/32 pre-scale
  applied at PSUM evacuation, undone in the IFFT constants)
  m_t[k] = sum_n cp_t[n,k] via bf16 ones-matmuls (fp32 PSUM accum)
  xi_t = irfft(m_t) via radix-64 Cooley-Tukey as tiny fp32 matmuls
  phi = l2norm(signed_sqrt([a0, a1*mean(x), a2*xi1, a3*xi2]))  all on device
"""
import sys
sys.path.insert(0, "/opt/trn_rl_repo")
from contextlib import ExitStack

import numpy as np
import ml_dtypes

from concourse import bass, tile, bacc, mybir
from concourse.bass_utils import run_bass_kernel_spmd

BF16 = mybir.dt.bfloat16
F32 = mybir.dt.float32
FP8 = mybir.dt.float8e4
AF = mybir.ActivationFunctionType
ALU = mybir.AluOpType
AX = mybir.AxisListType
PSUM = bass.MemorySpace.PSUM
PM = mybir.MatmulPerfMode

D = 4096
C = 512
B = 8
N = 784            # 28*28 positions per image
KF = 2049          # rfft bins
NT, NSZ = 7, 112   # position tiles
KW = 512           # max chunk width (bin 2048 handled separately)
CHUNKS = [(0, 512), (512, 512), (1024, 512), (1536, 512)]
NCH = len(CHUNKS)
SCALE = 1.0 / 32   # xf pre-scale so cp products fit fp8 range
EPS = 1e-12
NPHI = 1 + C + 2 * D  # 8705

_cache = {}


def _build_program(a0, a1):
    """Build the bass program. a0, a1 (floats) get baked in; array consts are inputs."""
    nc = bacc.Bacc("TRN2", target_bir_lowering=False, debug=False, num_devices=B)

    xb_d = nc.dram_tensor("xb", [N, C], BF16, kind="ExternalInput").ap()
    xp_d = nc.dram_tensor("xp8", [2, 128, 2, N], FP8, kind="ExternalInput").ap()
    A_d = nc.dram_tensor("Abig", [6, 2, 128, 2, 2048], FP8, kind="ExternalInput").ap()
    AN_d = nc.dram_tensor("Anyq", [128, 2, 2, 16], FP8, kind="ExternalInput").ap()
    W_d = nc.dram_tensor("Wc", [3, 32, 64], F32, kind="ExternalInput").ap()   # WR,WI,WnI
    CW_d = nc.dram_tensor("Cw", [4, 64, 64], F32, kind="ExternalInput").ap()  # CR1,CI1,CR2,CI2
    G_d = nc.dram_tensor("Gc", [2, 64, 64], F32, kind="ExternalInput").ap()   # GcosT,GnegsinT
    UV_d = nc.dram_tensor("uv", [4, 64], F32, kind="ExternalInput").ap()      # u1,v1,u2,v2

    mrow_d = [nc.dram_tensor(f"mrow{qi}", [1, KF], F32, kind="Internal").ap()
              for qi in range(4)]  # m1R,m1I,m2R,m2I
    phi0_d = nc.dram_tensor("phi0", [1, 1], F32, kind="ExternalOutput").ap()
    pfirst_d = nc.dram_tensor("pfirst", [C, 1], F32, kind="ExternalOutput").ap()
    pxi_d = [nc.dram_tensor(f"pxi{t}", [64, 64], F32, kind="ExternalOutput").ap()
             for t in (1, 2)]

    zsigned = float(np.sign(a0) * np.sqrt(abs(a0) + EPS))
    c0 = float(abs(a0) + NPHI * EPS)
    s1scale = float(a1 / N)
    s1sign = 1.0 if a1 >= 0 else -1.0

    with tile.TileContext(nc) as tc, ExitStack() as ctx:
        consts = ctx.enter_context(tc.tile_pool(name="consts", bufs=1))
        apool = ctx.enter_context(tc.tile_pool(name="apool", bufs=2))
        xpool = ctx.enter_context(tc.tile_pool(name="xp", bufs=1))
        stpool = ctx.enter_context(tc.tile_pool(name="stage", bufs=2))
        sfin = ctx.enter_context(tc.tile_pool(name="sfin", bufs=1))
        fin = ctx.enter_context(tc.tile_pool(name="fin", bufs=1))

        # ---- critical-path loads first: xp8 gates the first matmuls ----
        XP = []
        for pair in range(2):
            t = xpool.tile([128, 2, N], FP8, name=f"xp{pair}", tag=f"xp{pair}")
            nc.sync.dma_start(t[:, 0], xp_d[pair, :, 0])
            nc.scalar.dma_start(t[:, 1], xp_d[pair, :, 1])
            XP.append(t)
        AN_sb = xpool.tile([128, 2, 2, 16], FP8, name="anyq", tag="anyq")
        nc.sync.dma_start(AN_sb[:], AN_d[:])

        def load_a_tiles(ci, first):
            klo, kw = CHUNKS[ci]
            row = []
            for q in range(6):
                rr = []
                for pair in range(2):
                    t = apool.tile([128, 2, KW], FP8, name=f"a{q}{pair}",
                                   tag=f"a{q}{pair}")
                    if first:
                        eng = (nc.scalar, nc.sync, nc.gpsimd)[(2 * q + pair) % 3]
                    else:
                        eng = nc.gpsimd if (q + pair) % 2 == 0 else nc.sync
                    eng.dma_start(t[:, :, :kw], A_d[q, pair, :, :, klo:klo + kw])
                    rr.append(t)
                row.append(rr)
            return row

        a_pref = load_a_tiles(0, True)
        ones_bf = consts.tile([NSZ, 1], BF16, name="onesbf", tag="onesbf")
        nc.vector.memset(ones_bf[:], 1.0)

        # ---- non-critical loads, issued on idle queues at startup ----
        xb_sb = []
        for nt in range(NT):
            t = xpool.tile([NSZ, C], BF16, name=f"xb{nt}", tag=f"xb{nt}")
            nc.gpsimd.dma_start(t[:], xb_d[nt * NSZ:(nt + 1) * NSZ, :])
            xb_sb.append(t)
        ones1x64 = consts.tile([1, 64], F32, name="o1x64", tag="o1x64")
        nc.vector.memset(ones1x64[:], 1.0)
        ones1x128 = consts.tile([1, 128], F32, name="o1x128", tag="o1x128")
        nc.vector.memset(ones1x128[:], 1.0)
        onesP64 = consts.tile([64, 1], F32, name="oP64", tag="oP64")
        nc.vector.memset(onesP64[:], 1.0)
        onesP128 = consts.tile([128, 1], F32, name="oP128", tag="oP128")
        nc.vector.memset(onesP128[:], 1.0)
        eps128 = consts.tile([128, 1], F32, name="eps128", tag="eps128")
        nc.vector.memset(eps128[:], EPS)
        W_sb = []
        for i in range(3):
            t = consts.tile([32, 64], F32, name=f"W{i}", tag=f"W{i}")
            nc.gpsimd.dma_start(t[:], W_d[i])
            W_sb.append(t)
        CW_sb = []
        for i in range(4):
            t = consts.tile([64, 64], F32, name=f"CW{i}", tag=f"CW{i}")
            nc.gpsimd.dma_start(t[:], CW_d[i])
            CW_sb.append(t)
        G_sb = []
        for i in range(2):
            t = consts.tile([64, 64], F32, name=f"G{i}", tag=f"G{i}")
            nc.gpsimd.dma_start(t[:], G_d[i])
            G_sb.append(t)
        UV_sb = []
        for i in range(4):
            t = consts.tile([1, 64], F32, name=f"uv{i}", tag=f"uv{i}")
            nc.gpsimd.dma_start(t[:], UV_d[i:i + 1, :])
            UV_sb.append(t)

        # ---- Nyquist bin 2048: xfN[n, t] = sum_c x[n,c] s_t(c) (-1)^{h_t(c)}
        # (s_t pre-scaled by SCALE on host). Hidden under the chunk-0 A DMA.
        with tc.tile_pool(name="psN0", bufs=2, space=PSUM) as psN0, \
             tc.tile_pool(name="psN1", bufs=1, space=PSUM) as psN1, \
             tc.tile_pool(name="nyq", bufs=1) as nyq:
            cpn = nyq.tile([NSZ, 2 * NT], F32, name="cpn", tag="cpn")
            nyx = nyq.tile([NSZ, 3 * NT], F32, name="nyx", tag="nyx")
            for nt in range(NT):
                pn = psN0.tile([NSZ, 3], F32, name="pn", tag="pn")
                for pair in range(2):
                    nc.tensor.matmul(
                        pn[:], XP[pair][:, :, nt * NSZ:(nt + 1) * NSZ],
                        AN_sb[:, pair, :, 0:3],
                        start=(pair == 0), stop=(pair == 1),
                        perf_mode=PM.DoubleRow)
                xn = nyx[:, 3 * nt:3 * nt + 3]
                nc.vector.tensor_copy(xn, pn[:])
                # cp1N = xf0N*xf1N ; cp2N = cp1N*xf2N
                nc.vector.tensor_mul(cpn[:, 2 * nt:2 * nt + 1],
                                     xn[:, 0:1], xn[:, 1:2])
                nc.vector.tensor_mul(cpn[:, 2 * nt + 1:2 * nt + 2],
                                     cpn[:, 2 * nt:2 * nt + 1], xn[:, 2:3])
            ones32 = nyq.tile([NSZ, 1], F32, name="ones32", tag="ones32")
            nc.vector.memset(ones32[:], 1.0)
            pm = psN1.tile([1, 2 * NT], F32, name="pmn", tag="pmn")
            nc.tensor.matmul(pm[:], ones32[:], cpn[:], start=True, stop=True)
            mn = sfin.tile([1, 4], F32, name="mn", tag="mn")
            # m1R[2048], m2R[2048]; imag parts are zero
            nc.vector.tensor_reduce(
                mn[:, 0:1], pm[:].rearrange("p (a b) -> p a b", b=2)[:, :, 0],
                AX.X, ALU.add)
            nc.vector.tensor_reduce(
                mn[:, 2:3], pm[:].rearrange("p (a b) -> p a b", b=2)[:, :, 1],
                AX.X, ALU.add)
            nc.vector.memset(mn[:, 1:2], 0.0)
            nc.vector.memset(mn[:, 3:4], 0.0)
            for qi in (0, 2):
                nc.gpsimd.dma_start(mrow_d[qi][:, 2048:2049], mn[:, qi:qi + 1])


        # ---- first-order term early: rides the startup DMA window ----
        # first = a1 * mean_n x (per channel)
        absf, sgnf = [], []
        with tc.tile_pool(name="psF", bufs=4, space=PSUM) as psF:
            for ct in range(4):
                fp = psF.tile([128, 1], F32, name="fp", tag="fp")
                for nt in range(NT):
                    nc.tensor.matmul(
                        fp[:], xb_sb[nt][:, ct * 128:(ct + 1) * 128],
                        ones_bf[:],
                        start=(nt == 0), stop=(nt == NT - 1))
                av = sfin.tile([128, 1], F32, name=f"absf{ct}", tag=f"absf{ct}")
                nc.scalar.activation(av[:], fp[:], AF.Abs, scale=s1scale)
                sv = sfin.tile([128, 1], F32, name=f"sgnf{ct}", tag=f"sgnf{ct}")
                nc.scalar.activation(sv[:], fp[:], AF.Sign, scale=s1sign)
                absf.append(av)
                sgnf.append(sv)

        with tc.tile_pool(name="xfpool", bufs=2) as xfpool, \
             tc.tile_pool(name="cppool", bufs=2) as cppool, \
             tc.tile_pool(name="tmppool", bufs=1) as tmppool, \
             tc.tile_pool(name="psA", bufs=6, space=PSUM) as psA, \
             tc.tile_pool(name="psM", bufs=2, space=PSUM) as psM:

            pending = []

            def emit_mred(job):
                cpt, klo, kw = job
                last = klo + kw >= 2048
                for qi in range(4):
                    ps = psM.tile([1, KW], F32, name="psm", tag="psm")
                    for j in range(NT):
                        nc.tensor.matmul(
                            ps[:, :kw], ones_bf[:],
                            cpt[qi][:, j, :kw],
                            start=(j == 0), stop=(j == NT - 1))
                    st = stpool.tile([1, KW], F32, name="mstage", tag="mstage")
                    if last:
                        nc.scalar.copy(st[:, :kw], ps[:, :kw])
                    else:
                        nc.vector.tensor_copy(st[:, :kw], ps[:, :kw])
                    nc.sync.dma_start(
                        mrow_d[qi][:, klo:klo + kw], st[:, :kw])

            def emit_products_half(xf, cp, tA, tB, lo, hi, kw):
                # stage B on position tiles [lo, hi): cp1=xf0*xf1, cp2=cp1*xf2
                sl = (slice(None), slice(lo, hi), slice(0, kw))
                R0, I0, R1, I1, R2, I2 = (t[sl] for t in xf)
                cp1R, cp1I, cp2R, cp2I = (t[sl] for t in cp)
                a, b = tA[sl], tB[sl]
                nc.vector.tensor_mul(a, R0, R1)
                nc.vector.tensor_mul(b, I0, I1)
                nc.vector.tensor_sub(cp1R, a, b)
                nc.vector.tensor_mul(a, R0, I1)
                nc.vector.tensor_mul(b, I0, R1)
                nc.vector.tensor_add(cp1I, a, b)
                nc.vector.tensor_mul(a, cp1R, R2)
                nc.vector.tensor_mul(b, cp1I, I2)
                nc.vector.tensor_sub(cp2R, a, b)
                nc.vector.tensor_mul(a, cp1R, I2)
                nc.vector.tensor_mul(b, cp1I, R2)
                nc.vector.tensor_add(cp2I, a, b)

            for ci in range(NCH):
                klo, kw = CHUNKS[ci]
                a_sb = a_pref if ci == 0 else load_a_tiles(ci, False)

                # stage A: xf_q[n, k] = SCALE * sum_c x[n,c] A_q[c,k]
                xf = [xfpool.tile([NSZ, NT, KW], BF16, name=f"xf{q}",
                                  tag=f"xf{q}") for q in range(6)]
                cp = [cppool.tile([NSZ, NT, KW], BF16, name=f"cp{i}", tag=f"cp{i}")
                      for i in range(4)]
                tA = tmppool.tile([NSZ, NT, KW], BF16, name="tA", tag="tA")
                tB = tmppool.tile([NSZ, NT, KW], BF16, name="tB", tag="tB")
                for nt in range(NT):
                    for q in range(6):
                        ps = psA.tile([NSZ, KW], F32, name="psa", tag="psa")
                        for pair in range(2):
                            nc.tensor.matmul(
                                ps[:, :kw],
                                XP[pair][:, :, nt * NSZ:(nt + 1) * NSZ],
                                a_sb[q][pair][:, :, :kw],
                                start=(pair == 0), stop=(pair == 1),
                                perf_mode=PM.DoubleRow)
                        nc.scalar.activation(xf[q][:, nt, :kw], ps[:, :kw],
                                             AF.Copy, scale=SCALE)
                    if nt == 4:
                        # first-half products overlap this chunk's stage A
                        emit_products_half(xf, cp, tA, tB, 0, 5, kw)
                emit_products_half(xf, cp, tA, tB, 5, NT, kw)

                pending.append((cp, klo, kw))
                if ci >= 1:
                    emit_mred(pending.pop(0))
            while pending:
                emit_mred(pending.pop(0))

        # ================= final phase =================
        with tc.tile_pool(name="psT", bufs=1, space=PSUM) as psT, \
             tc.tile_pool(name="psY", bufs=1, space=PSUM) as psY, \
             tc.tile_pool(name="psN", bufs=1, space=PSUM) as psN, \
             tc.tile_pool(name="psB", bufs=1, space=PSUM) as psB:

            y_ps = []
            s_t = []
            mmTs = []
            for t in range(2):
                mmT = []
                for q in range(2):  # R, I
                    mt = fin.tile([32, 64], F32, name=f"mmT{t}{q}", tag=f"mmT{t}{q}")
                    nc.sync.dma_start(
                        mt[:],
                        mrow_d[2 * t + q][:, 0:2048]
                        .rearrange("p (a b) -> (p a) b", a=32))
                    mmT.append(mt)
                mmTs.append(mmT)
            m0s = [mmTs[0][0][0:1, 0:1], mmTs[1][0][0:1, 0:1]]
            mNs = [mn[:, 0:1], mn[:, 2:3]]
            TRs, TIs, Tps, crows = [], [], [], []
            for t in range(2):  # stage 1 DFT for both orders first
                mmT = mmTs[t]
                TR = psT.tile([64, 64], F32, name="TR", tag=f"TR{t}")
                nc.tensor.matmul(TR[:], mmT[0][:], W_sb[0][:], start=True, stop=False)
                nc.tensor.matmul(TR[:], mmT[1][:], W_sb[2][:], start=False, stop=True)
                TI = psT.tile([64, 64], F32, name="TI", tag=f"TI{t}")
                nc.tensor.matmul(TI[:], mmT[0][:], W_sb[1][:], start=True, stop=False)
                nc.tensor.matmul(TI[:], mmT[1][:], W_sb[0][:], start=False, stop=True)
                TRs.append(TR)
                TIs.append(TI)
            for t in range(2):  # twiddle + correction row (DVE)
                TR, TI = TRs[t], TIs[t]
                CR, CI = CW_sb[2 * t], CW_sb[2 * t + 1]
                ta = fin.tile([64, 64], F32, name=f"ta{t}", tag=f"ta{t}")
                tb = fin.tile([64, 64], F32, name=f"tb{t}", tag=f"tb{t}")
                TpR = fin.tile([64, 64], F32, name=f"TpR{t}", tag=f"TpR{t}")
                TpI = fin.tile([64, 64], F32, name=f"TpI{t}", tag=f"TpI{t}")
                nc.vector.tensor_mul(ta[:], TR[:], CR[:])
                nc.vector.tensor_mul(tb[:], TI[:], CI[:])
                nc.vector.tensor_sub(TpR[:], ta[:], tb[:])
                nc.vector.tensor_mul(ta[:], TR[:], CI[:])
                nc.vector.tensor_mul(tb[:], TI[:], CR[:])
                nc.vector.tensor_add(TpI[:], ta[:], tb[:])
                crow = fin.tile([1, 64], F32, name=f"crow{t}", tag=f"crow{t}")
                tmpr = fin.tile([1, 64], F32, name=f"tmpr{t}", tag=f"tmpr{t}")
                nc.vector.tensor_scalar_mul(tmpr[:], UV_sb[2 * t + 1][:], mNs[t])
                nc.vector.scalar_tensor_tensor(
                    crow[:], UV_sb[2 * t][:], m0s[t], tmpr[:],
                    op0=ALU.mult, op1=ALU.add)
                Tps.append((TpR, TpI))
                crows.append(crow)
            for t in range(2):  # stage 2 + correction broadcast
                TpR, TpI = Tps[t]
                y = psY.tile([64, 64], F32, name=f"y{t}", tag=f"y{t}")
                nc.tensor.matmul(y[:], G_sb[0][:], TpR[:], start=True, stop=False)
                nc.tensor.matmul(y[:], G_sb[1][:], TpI[:], start=False, stop=False)
                nc.tensor.matmul(y[:], ones1x64[:], crows[t][:], start=False,
                                 stop=True, skip_group_check=True)
                y_ps.append(y)
                st = fin.tile([64, 1], F32, name=f"st{t}", tag=f"st{t}")
                nc.vector.tensor_reduce(st[:], y[:], AX.X, ALU.add,
                                        apply_absolute_value=True)
                s_t.append(st)

            # norm total = sum|y1| + sum|y2| + sum|first| + (|a0| + NPHI*eps)
            tot = psN.tile([1, 1], F32, name="tot", tag="tot")
            nc.tensor.matmul(tot[:], onesP64[:], s_t[0][:], start=True, stop=False,
                             skip_group_check=True)
            nc.tensor.matmul(tot[:], onesP64[:], s_t[1][:], start=False, stop=False,
                             skip_group_check=True)
            for ct in range(4):
                nc.tensor.matmul(tot[:], onesP128[:], absf[ct][:],
                                 start=False, stop=(ct == 3),
                                 skip_group_check=True)
            tot_sb = fin.tile([1, 1], F32, name="tot_sb", tag="tot_sb")
            nc.scalar.activation(tot_sb[:], tot[:], AF.Copy, bias=c0)
            rec = fin.tile([1, 1], F32, name="rec", tag="rec")
            nc.vector.reciprocal(rec[:], tot_sb[:])
            ninv = fin.tile([1, 1], F32, name="ninv", tag="ninv")
            nc.scalar.sqrt(ninv[:], rec[:])
            nv128_ps = psB.tile([128, 1], F32, name="nv128", tag="nv128")
            nc.tensor.matmul(nv128_ps[:], ones1x128[:], ninv[:], start=True, stop=True)
            nv128 = fin.tile([128, 1], F32, name="nv128sb", tag="nv128sb")
            nc.scalar.copy(nv128[:], nv128_ps[:])
            nv64 = nv128[0:64]

            # phi pieces
            ph0 = fin.tile([1, 1], F32, name="ph0", tag="ph0")
            nc.vector.tensor_scalar_mul(ph0[:], ninv[:], zsigned)
            nc.sync.dma_start(phi0_d[:], ph0[:])
            for ct in range(4):
                sqf = fin.tile([128, 1], F32, name=f"sqf{ct}", tag=f"sqf{ct}")
                nc.scalar.activation(sqf[:], absf[ct][:], AF.Sqrt, bias=eps128[:])
                pmf = fin.tile([128, 1], F32, name=f"pmf{ct}", tag=f"pmf{ct}")
                nc.vector.tensor_mul(pmf[:], sqf[:], sgnf[ct][:])
                phf = fin.tile([128, 1], F32, name=f"phf{ct}", tag=f"phf{ct}")
                nc.vector.tensor_scalar_mul(phf[:], pmf[:], nv128[:])
                nc.sync.dma_start(pfirst_d[ct * 128:(ct + 1) * 128, :], phf[:])
            for t in range(2):
                ab = fin.tile([64, 64], F32, name=f"ab{t}", tag=f"ab{t}")
                nc.scalar.activation(ab[:], y_ps[t][:], AF.Abs)
                sq = fin.tile([64, 64], F32, name=f"sq{t}", tag=f"sq{t}")
                nc.scalar.activation(sq[:], ab[:], AF.Sqrt, bias=eps128[:64])
                sg = fin.tile([64, 64], F32, name=f"sg{t}", tag=f"sg{t}")
                nc.scalar.activation(sg[:], y_ps[t][:], AF.Sign)
                pm = fin.tile([64, 64], F32, name=f"pm{t}", tag=f"pm{t}")
                nc.vector.tensor_mul(pm[:], sq[:], sg[:])
                phx = fin.tile([64, 64], F32, name=f"phx{t}", tag=f"phx{t}")
                nc.vector.tensor_scalar_mul(phx[:], pm[:], nv64[:])
                nc.sync.dma_start(pxi_d[t][:], phx[:])

    nc.compile()
    return nc


def _host_prep(x, alpha, h_idx, s_bits):
    """Per-core input maps: fp8 image/DFT layouts + fp32 IFFT constants."""
    x = np.asarray(x, np.float32)
    alpha = np.asarray(alpha, np.float64)
    h_idx = np.asarray(h_idx).astype(np.int64)
    s_bits = np.asarray(s_bits).astype(np.int64)
    signs = (2 * s_bits - 1).astype(np.float64)

    # DFT matrices A_t[c, k] (fp8), packed for DoubleRow:
    # A8[q, pair, p, i, k] = A_q[pair*256 + i*128 + p, k]
    k = np.arange(2048, dtype=np.float64)[:, None]
    Abig = np.empty((6, C, 2048), ml_dtypes.float8_e4m3)
    for t in range(3):
        ang = -2.0 * np.pi * ((k * h_idx[t][None, :]) % D) / D
        Abig[2 * t] = (np.cos(ang) * signs[t][None, :]).T.astype(
            ml_dtypes.float8_e4m3)
        Abig[2 * t + 1] = (np.sin(ang) * signs[t][None, :]).T.astype(
            ml_dtypes.float8_e4m3)
    A8 = np.ascontiguousarray(
        Abig.reshape(6, 2, 2, 128, 2048).transpose(0, 1, 3, 2, 4))
    # AN8[p, pair, i, t] = SCALE * s_t(c) * (-1)^{h_t(c)},  c = pair*256+i*128+p
    Anyq = np.empty((C, 3), np.float64)
    for t in range(3):
        Anyq[:, t] = SCALE * signs[t] * ((-1.0) ** (h_idx[t] % 2))
    AN8 = np.zeros((128, 2, 2, 16), ml_dtypes.float8_e4m3)
    AN8[:, :, :, 0:3] = np.ascontiguousarray(
        Anyq.reshape(2, 2, 128, 3).transpose(2, 0, 1, 3)).astype(
        ml_dtypes.float8_e4m3)

    j0 = np.arange(64, dtype=np.float64)[None, :]
    k2 = np.arange(32, dtype=np.float64)[:, None]
    k1 = np.arange(64, dtype=np.float64)[:, None]
    Wc = np.empty((3, 32, 64), np.float32)
    Wc[0] = np.cos(2 * np.pi * k2 * j0 / 64)
    Wc[1] = np.sin(2 * np.pi * k2 * j0 / 64)
    Wc[2] = -Wc[1]
    Cw = np.empty((4, 64, 64), np.float32)
    uv = np.empty((4, 64), np.float32)
    for t in range(2):
        # undo the SCALE^(t+2) applied on-device to cp_{t+1}
        sig = 2.0 * alpha[2 + t] / (D * N) / SCALE ** (t + 2)
        Cw[2 * t] = sig * np.cos(2 * np.pi * k1 * j0 / D)
        Cw[2 * t + 1] = sig * np.sin(2 * np.pi * k1 * j0 / D)
        uv[2 * t] = -alpha[2 + t] / (D * N) / SCALE ** (t + 2)
        uv[2 * t + 1] = (alpha[2 + t] / (D * N) / SCALE ** (t + 2)
                         * ((-1.0) ** np.arange(64)))
    g = 2 * np.pi * k1 * np.arange(64)[None, :] / 64
    Gc = np.empty((2, 64, 64), np.float32)
    Gc[0] = np.cos(g)
    Gc[1] = -np.sin(g)

    in_maps = []
    xf = x.reshape(B, N, C)
    for b in range(B):
        # xp8[pair, p, i, n] = x[n, pair*256 + i*128 + p]
        xT = xf[b].T.reshape(2, 2, 128, N).transpose(0, 2, 1, 3)
        in_maps.append({
            "xb": xf[b].astype(ml_dtypes.bfloat16),
            "xp8": np.ascontiguousarray(xT).astype(ml_dtypes.float8_e4m3),
            "Abig": A8, "Anyq": AN8, "Wc": Wc, "Cw": Cw, "Gc": Gc, "uv": uv,
        })
    return in_maps, float(alpha[0]), float(alpha[1])


def kernel(x, alpha, h_idx, s_bits, _trace=False, _tmpdir=None):
    in_maps, a0, a1 = _host_prep(x, alpha, h_idx, s_bits)
    key = (round(a0, 12), round(a1, 12))
    if key not in _cache:
        _cache[key] = _build_program(a0, a1)
    nc = _cache[key]
    res = run_bass_kernel_spmd(nc, in_maps, core_ids=list(range(B)),
                               trace=_trace, tmpdir=_tmpdir)
    kernel.last_result = res
    out = np.empty((B, NPHI), np.float32)
    for b in range(B):
        r = res.results[b]
        out[b, 0] = r["phi0"][0, 0]
        out[b, 1:1 + C] = r["pfirst"].reshape(C)
        out[b, 1 + C:1 + C + D] = r["pxi1"].reshape(D)
        out[b, 1 + C + D:] = r["pxi2"].reshape(D)
    return out


# revision 35
# speedup vs baseline: 1.1833x; 1.0057x over previous
"""Trainium2 Bass kernel for KernelPooling (count-sketch polynomial pooling).

One image per NeuronCore (B=8 = n_cores). Per core:
  fft(count_sketch_t(x[n]))[k] = sum_c A_t[k,c] x[n,c] with
  A_t[k,c] = s_t(c)*exp(-2pi i k h_t(c)/D)  -> fp8 DoubleRow matmuls (PE,
  contraction 512 = 2 passes of 256)
  cp1 = xf0*xf1, cp2 = cp1*xf2 elementwise (DVE bf16; 1/32 pre-scale
  applied at PSUM evacuation, undone in the IFFT constants)
  m_t[k] = sum_n cp_t[n,k] via bf16 ones-matmuls (fp32 PSUM accum)
  xi_t = irfft(m_t) via radix-64 Cooley-Tukey as tiny fp32 matmuls
  phi = l2norm(signed_sqrt([a0, a1*mean(x), a2*xi1, a3*xi2]))  all on device
"""
import sys
sys.path.insert(0, "/opt/trn_rl_repo")
from contextlib import ExitStack

import numpy as np
import ml_dtypes

from concourse import bass, tile, bacc, mybir
from concourse.bass_utils import run_bass_kernel_spmd

BF16 = mybir.dt.bfloat16
F32 = mybir.dt.float32
FP8 = mybir.dt.float8e4
AF = mybir.ActivationFunctionType
ALU = mybir.AluOpType
AX = mybir.AxisListType
PSUM = bass.MemorySpace.PSUM
PM = mybir.MatmulPerfMode

D = 4096
C = 512
B = 8
N = 784            # 28*28 positions per image
KF = 2049          # rfft bins
NT, NSZ = 7, 112   # position tiles
KW = 512           # max chunk width (bin 2048 handled separately)
CHUNKS = [(0, 512), (512, 512), (1024, 512), (1536, 512)]
NCH = len(CHUNKS)
SCALE = 1.0 / 32   # xf pre-scale so cp products fit fp8 range
EPS = 1e-12
NPHI = 1 + C + 2 * D  # 8705

_cache = {}


def _build_program(a0, a1):
    """Build the bass program. a0, a1 (floats) get baked in; array consts are inputs."""
    nc = bacc.Bacc("TRN2", target_bir_lowering=False, debug=False, num_devices=B)

    xb_d = nc.dram_tensor("xb", [N, C], BF16, kind="ExternalInput").ap()
    xp_d = nc.dram_tensor("xp8", [2, 128, 2, N], FP8, kind="ExternalInput").ap()
    A_d = nc.dram_tensor("Abig", [6, 2, 128, 2, 2048], FP8, kind="ExternalInput").ap()
    AN_d = nc.dram_tensor("Anyq", [128, 2, 2, 16], FP8, kind="ExternalInput").ap()
    W_d = nc.dram_tensor("Wc", [3, 32, 64], F32, kind="ExternalInput").ap()   # WR,WI,WnI
    CW_d = nc.dram_tensor("Cw", [4, 64, 64], F32, kind="ExternalInput").ap()  # CR1,CI1,CR2,CI2
    G_d = nc.dram_tensor("Gc", [2, 64, 64], F32, kind="ExternalInput").ap()   # GcosT,GnegsinT
    UV_d = nc.dram_tensor("uv", [4, 64], F32, kind="ExternalInput").ap()      # u1,v1,u2,v2

    mrow_d = [nc.dram_tensor(f"mrow{qi}", [1, KF], F32, kind="Internal").ap()
              for qi in range(4)]  # m1R,m1I,m2R,m2I
    phi0_d = nc.dram_tensor("phi0", [1, 1], F32, kind="ExternalOutput").ap()
    pfirst_d = nc.dram_tensor("pfirst", [C, 1], F32, kind="ExternalOutput").ap()
    pxi_d = [nc.dram_tensor(f"pxi{t}", [64, 64], F32, kind="ExternalOutput").ap()
             for t in (1, 2)]

    zsigned = float(np.sign(a0) * np.sqrt(abs(a0) + EPS))
    c0 = float(abs(a0) + NPHI * EPS)
    s1scale = float(a1 / N)
    s1sign = 1.0 if a1 >= 0 else -1.0

    with tile.TileContext(nc) as tc, ExitStack() as ctx:
        consts = ctx.enter_context(tc.tile_pool(name="consts", bufs=1))
        apool = ctx.enter_context(tc.tile_pool(name="apool", bufs=2))
        xpool = ctx.enter_context(tc.tile_pool(name="xp", bufs=1))
        stpool = ctx.enter_context(tc.tile_pool(name="stage", bufs=2))
        sfin = ctx.enter_context(tc.tile_pool(name="sfin", bufs=1))
        fin = ctx.enter_context(tc.tile_pool(name="fin", bufs=1))

        # ---- critical-path loads first: xp8 gates the first matmuls ----
        XP = []
        for pair in range(2):
            t = xpool.tile([128, 2, N], FP8, name=f"xp{pair}", tag=f"xp{pair}")
            nc.sync.dma_start(t[:, 0], xp_d[pair, :, 0])
            nc.scalar.dma_start(t[:, 1], xp_d[pair, :, 1])
            XP.append(t)
        AN_sb = xpool.tile([128, 2, 2, 16], FP8, name="anyq", tag="anyq")
        nc.sync.dma_start(AN_sb[:], AN_d[:])

        def load_a_tiles(ci, first):
            klo, kw = CHUNKS[ci]
            row = []
            for q in range(6):
                rr = []
                for pair in range(2):
                    t = apool.tile([128, 2, KW], FP8, name=f"a{q}{pair}",
                                   tag=f"a{q}{pair}")
                    if first:
                        eng = (nc.scalar, nc.sync, nc.gpsimd)[(2 * q + pair) % 3]
                    else:
                        eng = nc.gpsimd if (q + pair) % 2 == 0 else nc.sync
                    eng.dma_start(t[:, :, :kw], A_d[q, pair, :, :, klo:klo + kw])
                    rr.append(t)
                row.append(rr)
            return row

        a_pref = load_a_tiles(0, True)
        ones_bf = consts.tile([NSZ, 1], BF16, name="onesbf", tag="onesbf")
        nc.vector.memset(ones_bf[:], 1.0)

        # ---- non-critical loads, issued on idle queues at startup ----
        xb_sb = []
        for nt in range(NT):
            t = xpool.tile([NSZ, C], BF16, name=f"xb{nt}", tag=f"xb{nt}")
            nc.gpsimd.dma_start(t[:], xb_d[nt * NSZ:(nt + 1) * NSZ, :])
            xb_sb.append(t)
        ones1x64 = consts.tile([1, 64], F32, name="o1x64", tag="o1x64")
        nc.vector.memset(ones1x64[:], 1.0)
        ones1x128 = consts.tile([1, 128], F32, name="o1x128", tag="o1x128")
        nc.vector.memset(ones1x128[:], 1.0)
        onesP64 = consts.tile([64, 1], F32, name="oP64", tag="oP64")
        nc.vector.memset(onesP64[:], 1.0)
        onesP128 = consts.tile([128, 1], F32, name="oP128", tag="oP128")
        nc.vector.memset(onesP128[:], 1.0)
        eps128 = consts.tile([128, 1], F32, name="eps128", tag="eps128")
        nc.vector.memset(eps128[:], EPS)
        W_sb = []
        for i in range(3):
            t = consts.tile([32, 64], F32, name=f"W{i}", tag=f"W{i}")
            nc.gpsimd.dma_start(t[:], W_d[i])
            W_sb.append(t)
        CW_sb = []
        for i in range(4):
            t = consts.tile([64, 64], F32, name=f"CW{i}", tag=f"CW{i}")
            nc.gpsimd.dma_start(t[:], CW_d[i])
            CW_sb.append(t)
        G_sb = []
        for i in range(2):
            t = consts.tile([64, 64], F32, name=f"G{i}", tag=f"G{i}")
            nc.gpsimd.dma_start(t[:], G_d[i])
            G_sb.append(t)
        UV_sb = []
        for i in range(4):
            t = consts.tile([1, 64], F32, name=f"uv{i}", tag=f"uv{i}")
            nc.gpsimd.dma_start(t[:], UV_d[i:i + 1, :])
            UV_sb.append(t)

        # ---- Nyquist bin 2048: xfN[n, t] = sum_c x[n,c] s_t(c) (-1)^{h_t(c)}
        # (s_t pre-scaled by SCALE on host). Hidden under the chunk-0 A DMA.
        with tc.tile_pool(name="psN0", bufs=2, space=PSUM) as psN0, \
             tc.tile_pool(name="psN1", bufs=1, space=PSUM) as psN1, \
             tc.tile_pool(name="nyq", bufs=1) as nyq:
            cpn = nyq.tile([NSZ, 2 * NT], F32, name="cpn", tag="cpn")
            nyx = nyq.tile([NSZ, 3 * NT], F32, name="nyx", tag="nyx")
            for nt in range(NT):
                pn = psN0.tile([NSZ, 3], F32, name="pn", tag="pn")
                for pair in range(2):
                    nc.tensor.matmul(
                        pn[:], XP[pair][:, :, nt * NSZ:(nt + 1) * NSZ],
                        AN_sb[:, pair, :, 0:3],
                        start=(pair == 0), stop=(pair == 1),
                        perf_mode=PM.DoubleRow)
                xn = nyx[:, 3 * nt:3 * nt + 3]
                nc.vector.tensor_copy(xn, pn[:])
                # cp1N = xf0N*xf1N ; cp2N = cp1N*xf2N
                nc.vector.tensor_mul(cpn[:, 2 * nt:2 * nt + 1],
                                     xn[:, 0:1], xn[:, 1:2])
                nc.vector.tensor_mul(cpn[:, 2 * nt + 1:2 * nt + 2],
                                     cpn[:, 2 * nt:2 * nt + 1], xn[:, 2:3])
            ones32 = nyq.tile([NSZ, 1], F32, name="ones32", tag="ones32")
            nc.vector.memset(ones32[:], 1.0)
            pm = psN1.tile([1, 2 * NT], F32, name="pmn", tag="pmn")
            nc.tensor.matmul(pm[:], ones32[:], cpn[:], start=True, stop=True)
            mn = sfin.tile([1, 4], F32, name="mn", tag="mn")
            # m1R[2048], m2R[2048]; imag parts are zero
            nc.vector.tensor_reduce(
                mn[:, 0:1], pm[:].rearrange("p (a b) -> p a b", b=2)[:, :, 0],
                AX.X, ALU.add)
            nc.vector.tensor_reduce(
                mn[:, 2:3], pm[:].rearrange("p (a b) -> p a b", b=2)[:, :, 1],
                AX.X, ALU.add)
            nc.vector.memset(mn[:, 1:2], 0.0)
            nc.vector.memset(mn[:, 3:4], 0.0)
            for qi in (0, 2):
                nc.gpsimd.dma_start(mrow_d[qi][:, 2048:2049], mn[:, qi:qi + 1])


        # ---- first-order term early: rides the startup DMA window ----
        # first = a1 * mean_n x (per channel)
        absf, sgnf = [], []
        with tc.tile_pool(name="psF", bufs=4, space=PSUM) as psF:
            for ct in range(4):
                fp = psF.tile([128, 1], F32, name="fp", tag="fp")
                for nt in range(NT):
                    nc.tensor.matmul(
                        fp[:], xb_sb[nt][:, ct * 128:(ct + 1) * 128],
                        ones_bf[:],
                        start=(nt == 0), stop=(nt == NT - 1))
                av = sfin.tile([128, 1], F32, name=f"absf{ct}", tag=f"absf{ct}")
                nc.scalar.activation(av[:], fp[:], AF.Abs, scale=s1scale)
                sv = sfin.tile([128, 1], F32, name=f"sgnf{ct}", tag=f"sgnf{ct}")
                nc.scalar.activation(sv[:], fp[:], AF.Sign, scale=s1sign)
                absf.append(av)
                sgnf.append(sv)

        with tc.tile_pool(name="xfpool", bufs=2) as xfpool, \
             tc.tile_pool(name="cppool", bufs=2) as cppool, \
             tc.tile_pool(name="tmppool", bufs=1) as tmppool, \
             tc.tile_pool(name="psA", bufs=3, space=PSUM) as psA, \
             tc.tile_pool(name="psM", bufs=2, space=PSUM) as psM:

            pending = []

            def emit_mred(job):
                cpt, klo, kw = job
                last = klo + kw >= 2048
                for qi in range(4):
                    ps = psM.tile([1, KW], F32, name="psm", tag="psm")
                    for j in range(NT):
                        nc.tensor.matmul(
                            ps[:, :kw], ones_bf[:],
                            cpt[qi][:, j, :kw],
                            start=(j == 0), stop=(j == NT - 1))
                    st = stpool.tile([1, KW], F32, name="mstage", tag="mstage")
                    if last:
                        nc.scalar.copy(st[:, :kw], ps[:, :kw])
                    else:
                        nc.vector.tensor_copy(st[:, :kw], ps[:, :kw])
                    nc.sync.dma_start(
                        mrow_d[qi][:, klo:klo + kw], st[:, :kw])

            def emit_products_half(xf, cp, tA, tB, lo, hi, kw):
                # stage B on position tiles [lo, hi): cp1=xf0*xf1, cp2=cp1*xf2
                sl = (slice(None), slice(lo, hi), slice(0, kw))
                R0, I0, R1, I1, R2, I2 = (t[sl] for t in xf)
                cp1R, cp1I, cp2R, cp2I = (t[sl] for t in cp)
                a, b = tA[sl], tB[sl]
                nc.vector.tensor_mul(a, R0, R1)
                nc.vector.tensor_mul(b, I0, I1)
                nc.vector.tensor_sub(cp1R, a, b)
                nc.vector.tensor_mul(a, R0, I1)
                nc.vector.tensor_mul(b, I0, R1)
                nc.vector.tensor_add(cp1I, a, b)
                nc.vector.tensor_mul(a, cp1R, R2)
                nc.vector.tensor_mul(b, cp1I, I2)
                nc.vector.tensor_sub(cp2R, a, b)
                nc.vector.tensor_mul(a, cp1R, I2)
                nc.vector.tensor_mul(b, cp1I, R2)
                nc.vector.tensor_add(cp2I, a, b)

            for ci in range(NCH):
                klo, kw = CHUNKS[ci]
                a_sb = a_pref if ci == 0 else load_a_tiles(ci, False)

                # stage A: xf_q[n, k] = SCALE * sum_c x[n,c] A_q[c,k]
                # q-pairs share one [NSZ, 2, KW] psum tile -> one evac copy
                xfg = [xfpool.tile([NSZ, NT, 2, KW], BF16, name=f"xfg{g}",
                                   tag=f"xfg{g}") for g in range(3)]
                xf = [xfg[q // 2][:, :, q % 2, :] for q in range(6)]
                cp = [cppool.tile([NSZ, NT, KW], BF16, name=f"cp{i}", tag=f"cp{i}")
                      for i in range(4)]
                tA = tmppool.tile([NSZ, NT, KW], BF16, name="tA", tag="tA")
                tB = tmppool.tile([NSZ, NT, KW], BF16, name="tB", tag="tB")
                for nt in range(NT):
                    for g in range(3):
                        ps = psA.tile([NSZ, 2, KW], F32, name="psa", tag="psa")
                        for qq in range(2):
                            for pair in range(2):
                                nc.tensor.matmul(
                                    ps[:, qq, :kw],
                                    XP[pair][:, :, nt * NSZ:(nt + 1) * NSZ],
                                    a_sb[2 * g + qq][pair][:, :, :kw],
                                    start=(pair == 0), stop=(pair == 1),
                                    perf_mode=PM.DoubleRow)
                        nc.scalar.activation(xfg[g][:, nt, :, :kw], ps[:, :, :kw],
                                             AF.Copy, scale=SCALE)
                    if nt == 4:
                        # first-half products overlap this chunk's stage A
                        emit_products_half(xf, cp, tA, tB, 0, 5, kw)
                emit_products_half(xf, cp, tA, tB, 5, NT, kw)

                pending.append((cp, klo, kw))
                if ci >= 1:
                    emit_mred(pending.pop(0))
            while pending:
                emit_mred(pending.pop(0))

        # ================= final phase =================
        with tc.tile_pool(name="psT", bufs=1, space=PSUM) as psT, \
             tc.tile_pool(name="psY", bufs=1, space=PSUM) as psY, \
             tc.tile_pool(name="psN", bufs=1, space=PSUM) as psN, \
             tc.tile_pool(name="psB", bufs=1, space=PSUM) as psB:

            y_ps = []
            s_t = []
            mmTs = []
            for t in range(2):
                mmT = []
                for q in range(2):  # R, I
                    mt = fin.tile([32, 64], F32, name=f"mmT{t}{q}", tag=f"mmT{t}{q}")
                    nc.sync.dma_start(
                        mt[:],
                        mrow_d[2 * t + q][:, 0:2048]
                        .rearrange("p (a b) -> (p a) b", a=32))
                    mmT.append(mt)
                mmTs.append(mmT)
            m0s = [mmTs[0][0][0:1, 0:1], mmTs[1][0][0:1, 0:1]]
            mNs = [mn[:, 0:1], mn[:, 2:3]]
            TRs, TIs, Tps, crows = [], [], [], []
            for t in range(2):  # stage 1 DFT for both orders first
                mmT = mmTs[t]
                TR = psT.tile([64, 64], F32, name="TR", tag=f"TR{t}")
                nc.tensor.matmul(TR[:], mmT[0][:], W_sb[0][:], start=True, stop=False)
                nc.tensor.matmul(TR[:], mmT[1][:], W_sb[2][:], start=False, stop=True)
                TI = psT.tile([64, 64], F32, name="TI", tag=f"TI{t}")
                nc.tensor.matmul(TI[:], mmT[0][:], W_sb[1][:], start=True, stop=False)
                nc.tensor.matmul(TI[:], mmT[1][:], W_sb[0][:], start=False, stop=True)
                TRs.append(TR)
                TIs.append(TI)
            for t in range(2):  # twiddle + correction row (DVE)
                TR, TI = TRs[t], TIs[t]
                CR, CI = CW_sb[2 * t], CW_sb[2 * t + 1]
                ta = fin.tile([64, 64], F32, name=f"ta{t}", tag=f"ta{t}")
                tb = fin.tile([64, 64], F32, name=f"tb{t}", tag=f"tb{t}")
                TpR = fin.tile([64, 64], F32, name=f"TpR{t}", tag=f"TpR{t}")
                TpI = fin.tile([64, 64], F32, name=f"TpI{t}", tag=f"TpI{t}")
                nc.vector.tensor_mul(ta[:], TR[:], CR[:])
                nc.vector.tensor_mul(tb[:], TI[:], CI[:])
                nc.vector.tensor_sub(TpR[:], ta[:], tb[:])
                nc.vector.tensor_mul(ta[:], TR[:], CI[:])
                nc.vector.tensor_mul(tb[:], TI[:], CR[:])
                nc.vector.tensor_add(TpI[:], ta[:], tb[:])
                crow = fin.tile([1, 64], F32, name=f"crow{t}", tag=f"crow{t}")
                tmpr = fin.tile([1, 64], F32, name=f"tmpr{t}", tag=f"tmpr{t}")
                nc.vector.tensor_scalar_mul(tmpr[:], UV_sb[2 * t + 1][:], mNs[t])
                nc.vector.scalar_tensor_tensor(
                    crow[:], UV_sb[2 * t][:], m0s[t], tmpr[:],
                    op0=ALU.mult, op1=ALU.add)
                Tps.append((TpR, TpI))
                crows.append(crow)
            for t in range(2):  # stage 2 + correction broadcast
                TpR, TpI = Tps[t]
                y = psY.tile([64, 64], F32, name=f"y{t}", tag=f"y{t}")
                nc.tensor.matmul(y[:], G_sb[0][:], TpR[:], start=True, stop=False)
                nc.tensor.matmul(y[:], G_sb[1][:], TpI[:], start=False, stop=False)
                nc.tensor.matmul(y[:], ones1x64[:], crows[t][:], start=False,
                                 stop=True, skip_group_check=True)
                y_ps.append(y)
                st = fin.tile([64, 1], F32, name=f"st{t}", tag=f"st{t}")
                nc.vector.tensor_reduce(st[:], y[:], AX.X, ALU.add,
                                        apply_absolute_value=True)
                s_t.append(st)

            # norm total = sum|y1| + sum|y2| + sum|first| + (|a0| + NPHI*eps)
            tot = psN.tile([1, 1], F32, name="tot", tag="tot")
            nc.tensor.matmul(tot[:], onesP64[:], s_t[0][:], start=True, stop=False,
                             skip_group_check=True)
            nc.tensor.matmul(tot[:], onesP64[:], s_t[1][:], start=False, stop=False,
                             skip_group_check=True)
            for ct in range(4):
                nc.tensor.matmul(tot[:], onesP128[:], absf[ct][:],
                                 start=False, stop=(ct == 3),
                                 skip_group_check=True)
            tot_sb = fin.tile([1, 1], F32, name="tot_sb", tag="tot_sb")
            nc.scalar.activation(tot_sb[:], tot[:], AF.Copy, bias=c0)
            rec = fin.tile([1, 1], F32, name="rec", tag="rec")
            nc.vector.reciprocal(rec[:], tot_sb[:])
            ninv = fin.tile([1, 1], F32, name="ninv", tag="ninv")
            nc.scalar.sqrt(ninv[:], rec[:])
            nv128_ps = psB.tile([128, 1], F32, name="nv128", tag="nv128")
            nc.tensor.matmul(nv128_ps[:], ones1x128[:], ninv[:], start=True, stop=True)
            nv128 = fin.tile([128, 1], F32, name="nv128sb", tag="nv128sb")
            nc.scalar.copy(nv128[:], nv128_ps[:])
            nv64 = nv128[0:64]

            # phi pieces
            ph0 = fin.tile([1, 1], F32, name="ph0", tag="ph0")
            nc.vector.tensor_scalar_mul(ph0[:], ninv[:], zsigned)
            nc.sync.dma_start(phi0_d[:], ph0[:])
            for ct in range(4):
                sqf = fin.tile([128, 1], F32, name=f"sqf{ct}", tag=f"sqf{ct}")
                nc.scalar.activation(sqf[:], absf[ct][:], AF.Sqrt, bias=eps128[:])
                pmf = fin.tile([128, 1], F32, name=f"pmf{ct}", tag=f"pmf{ct}")
                nc.vector.tensor_mul(pmf[:], sqf[:], sgnf[ct][:])
                phf = fin.tile([128, 1], F32, name=f"phf{ct}", tag=f"phf{ct}")
                nc.vector.tensor_scalar_mul(phf[:], pmf[:], nv128[:])
                nc.sync.dma_start(pfirst_d[ct * 128:(ct + 1) * 128, :], phf[:])
            for t in range(2):
                ab = fin.tile([64, 64], F32, name=f"ab{t}", tag=f"ab{t}")
                nc.scalar.activation(ab[:], y_ps[t][:], AF.Abs)
                sq = fin.tile([64, 64], F32, name=f"sq{t}", tag=f"sq{t}")
                nc.scalar.activation(sq[:], ab[:], AF.Sqrt, bias=eps128[:64])
                sg = fin.tile([64, 64], F32, name=f"sg{t}", tag=f"sg{t}")
                nc.scalar.activation(sg[:], y_ps[t][:], AF.Sign)
                pm = fin.tile([64, 64], F32, name=f"pm{t}", tag=f"pm{t}")
                nc.vector.tensor_mul(pm[:], sq[:], sg[:])
                phx = fin.tile([64, 64], F32, name=f"phx{t}", tag=f"phx{t}")
                nc.vector.tensor_scalar_mul(phx[:], pm[:], nv64[:])
                nc.sync.dma_start(pxi_d[t][:], phx[:])

    nc.compile()
    return nc


def _host_prep(x, alpha, h_idx, s_bits):
    """Per-core input maps: fp8 image/DFT layouts + fp32 IFFT constants."""
    x = np.asarray(x, np.float32)
    alpha = np.asarray(alpha, np.float64)
    h_idx = np.asarray(h_idx).astype(np.int64)
    s_bits = np.asarray(s_bits).astype(np.int64)
    signs = (2 * s_bits - 1).astype(np.float64)

    # DFT matrices A_t[c, k] (fp8), packed for DoubleRow:
    # A8[q, pair, p, i, k] = A_q[pair*256 + i*128 + p, k]
    k = np.arange(2048, dtype=np.float64)[:, None]
    Abig = np.empty((6, C, 2048), ml_dtypes.float8_e4m3)
    for t in range(3):
        ang = -2.0 * np.pi * ((k * h_idx[t][None, :]) % D) / D
        Abig[2 * t] = (np.cos(ang) * signs[t][None, :]).T.astype(
            ml_dtypes.float8_e4m3)
        Abig[2 * t + 1] = (np.sin(ang) * signs[t][None, :]).T.astype(
            ml_dtypes.float8_e4m3)
    A8 = np.ascontiguousarray(
        Abig.reshape(6, 2, 2, 128, 2048).transpose(0, 1, 3, 2, 4))
    # AN8[p, pair, i, t] = SCALE * s_t(c) * (-1)^{h_t(c)},  c = pair*256+i*128+p
    Anyq = np.empty((C, 3), np.float64)
    for t in range(3):
        Anyq[:, t] = SCALE * signs[t] * ((-1.0) ** (h_idx[t] % 2))
    AN8 = np.zeros((128, 2, 2, 16), ml_dtypes.float8_e4m3)
    AN8[:, :, :, 0:3] = np.ascontiguousarray(
        Anyq.reshape(2, 2, 128, 3).transpose(2, 0, 1, 3)).astype(
        ml_dtypes.float8_e4m3)

    j0 = np.arange(64, dtype=np.float64)[None, :]
    k2 = np.arange(32, dtype=np.float64)[:, None]
    k1 = np.arange(64, dtype=np.float64)[:, None]
    Wc = np.empty((3, 32, 64), np.float32)
    Wc[0] = np.cos(2 * np.pi * k2 * j0 / 64)
    Wc[1] = np.sin(2 * np.pi * k2 * j0 / 64)
    Wc[2] = -Wc[1]
    Cw = np.empty((4, 64, 64), np.float32)
    uv = np.empty((4, 64), np.float32)
    for t in range(2):
        # undo the SCALE^(t+2) applied on-device to cp_{t+1}
        sig = 2.0 * alpha[2 + t] / (D * N) / SCALE ** (t + 2)
        Cw[2 * t] = sig * np.cos(2 * np.pi * k1 * j0 / D)
        Cw[2 * t + 1] = sig * np.sin(2 * np.pi * k1 * j0 / D)
        uv[2 * t] = -alpha[2 + t] / (D * N) / SCALE ** (t + 2)
        uv[2 * t + 1] = (alpha[2 + t] / (D * N) / SCALE ** (t + 2)
                         * ((-1.0) ** np.arange(64)))
    g = 2 * np.pi * k1 * np.arange(64)[None, :] / 64
    Gc = np.empty((2, 64, 64), np.float32)
    Gc[0] = np.cos(g)
    Gc[1] = -np.sin(g)

    in_maps = []
    xf = x.reshape(B, N, C)
    for b in range(B):
        # xp8[pair, p, i, n] = x[n, pair*256 + i*128 + p]
        xT = xf[b].T.reshape(2, 2, 128, N).transpose(0, 2, 1, 3)
        in_maps.append({
            "xb": xf[b].astype(ml_dtypes.bfloat16),
            "xp8": np.ascontiguousarray(xT).astype(ml_dtypes.float8_e4m3),
            "Abig": A8, "Anyq": AN8, "Wc": Wc, "Cw": Cw, "Gc": Gc, "uv": uv,
        })
    return in_maps, float(alpha[0]), float(alpha[1])


def kernel(x, alpha, h_idx, s_bits, _trace=False, _tmpdir=None):
    in_maps, a0, a1 = _host_prep(x, alpha, h_idx, s_bits)
    key = (round(a0, 12), round(a1, 12))
    if key not in _cache:
        _cache[key] = _build_program(a0, a1)
    nc = _cache[key]
    res = run_bass_kernel_spmd(nc, in_maps, core_ids=list(range(B)),
                               trace=_trace, tmpdir=_tmpdir)
    kernel.last_result = res
    out = np.empty((B, NPHI), np.float32)
    for b in range(B):
        r = res.results[b]
        out[b, 0] = r["phi0"][0, 0]
        out[b, 1:1 + C] = r["pfirst"].reshape(C)
        out[b, 1 + C:1 + C + D] = r["pxi1"].reshape(D)
        out[b, 1 + C + D:] = r["pxi2"].reshape(D)
    return out


# revision 36
# speedup vs baseline: 1.2677x; 1.0713x over previous
"""Trainium2 Bass kernel for KernelPooling (count-sketch polynomial pooling).

One image per NeuronCore (B=8 = n_cores). Per core:
  fft(count_sketch_t(x[n]))[k] = sum_c A_t[k,c] x[n,c] with
  A_t[k,c] = s_t(c)*exp(-2pi i k h_t(c)/D)  -> fp8 DoubleRow matmuls (PE,
  contraction 512 = 2 passes of 256)
  cp1 = xf0*xf1, cp2 = cp1*xf2 elementwise (DVE bf16; 1/32 pre-scale
  applied at PSUM evacuation, undone in the IFFT constants)
  m_t[k] = sum_n cp_t[n,k] via bf16 ones-matmuls (fp32 PSUM accum)
  xi_t = irfft(m_t) via radix-64 Cooley-Tukey as tiny fp32 matmuls
  phi = l2norm(signed_sqrt([a0, a1*mean(x), a2*xi1, a3*xi2]))  all on device
"""
import sys
sys.path.insert(0, "/opt/trn_rl_repo")
from contextlib import ExitStack

import numpy as np
import ml_dtypes

from concourse import bass, tile, bacc, mybir
from concourse.bass_utils import run_bass_kernel_spmd

BF16 = mybir.dt.bfloat16
F32 = mybir.dt.float32
FP8 = mybir.dt.float8e4
AF = mybir.ActivationFunctionType
ALU = mybir.AluOpType
AX = mybir.AxisListType
PSUM = bass.MemorySpace.PSUM
PM = mybir.MatmulPerfMode

D = 4096
C = 512
B = 8
N = 784            # 28*28 positions per image
KF = 2049          # rfft bins
NT, NSZ = 7, 112   # position tiles
KW = 512           # max chunk width (bin 2048 handled separately)
CHUNKS = [(0, 512), (512, 512), (1024, 512), (1536, 512)]
NCH = len(CHUNKS)
SCALE = 1.0 / 32   # xf pre-scale so cp products fit fp8 range
EPS = 1e-12
NPHI = 1 + C + 2 * D  # 8705

_cache = {}


def _build_program(a0, a1):
    """Build the bass program. a0, a1 (floats) get baked in; array consts are inputs."""
    nc = bacc.Bacc("TRN2", target_bir_lowering=False, debug=False, num_devices=B)

    xb_d = nc.dram_tensor("xb", [N, C], BF16, kind="ExternalInput").ap()
    xp_d = nc.dram_tensor("xp8", [2, 128, 2, N], FP8, kind="ExternalInput").ap()
    A_d = nc.dram_tensor("Abig", [6, 2, 128, 2, 2048], FP8, kind="ExternalInput").ap()
    AN_d = nc.dram_tensor("Anyq", [128, 2, 2, 16], FP8, kind="ExternalInput").ap()
    W_d = nc.dram_tensor("Wc", [3, 32, 64], F32, kind="ExternalInput").ap()   # WR,WI,WnI
    CW_d = nc.dram_tensor("Cw", [4, 64, 64], F32, kind="ExternalInput").ap()  # CR1,CI1,CR2,CI2
    G_d = nc.dram_tensor("Gc", [2, 64, 64], F32, kind="ExternalInput").ap()   # GcosT,GnegsinT
    UV_d = nc.dram_tensor("uv", [4, 64], F32, kind="ExternalInput").ap()      # u1,v1,u2,v2

    mrow_d = [nc.dram_tensor(f"mrow{qi}", [1, KF], F32, kind="Internal").ap()
              for qi in range(4)]  # m1R,m1I,m2R,m2I
    phi0_d = nc.dram_tensor("phi0", [1, 1], F32, kind="ExternalOutput").ap()
    pfirst_d = nc.dram_tensor("pfirst", [C, 1], F32, kind="ExternalOutput").ap()
    pxi_d = [nc.dram_tensor(f"pxi{t}", [64, 64], F32, kind="ExternalOutput").ap()
             for t in (1, 2)]

    zsigned = float(np.sign(a0) * np.sqrt(abs(a0) + EPS))
    c0 = float(abs(a0) + NPHI * EPS)
    s1scale = float(a1 / N)
    s1sign = 1.0 if a1 >= 0 else -1.0

    with tile.TileContext(nc) as tc, ExitStack() as ctx:
        consts = ctx.enter_context(tc.tile_pool(name="consts", bufs=1))
        apool = ctx.enter_context(tc.tile_pool(name="apool", bufs=2))
        xpool = ctx.enter_context(tc.tile_pool(name="xp", bufs=1))
        stpool = ctx.enter_context(tc.tile_pool(name="stage", bufs=2))
        sfin = ctx.enter_context(tc.tile_pool(name="sfin", bufs=1))
        fin = ctx.enter_context(tc.tile_pool(name="fin", bufs=1))

        # ---- critical-path loads first: xp8 gates the first matmuls ----
        XP = []
        for pair in range(2):
            t = xpool.tile([128, 2, N], FP8, name=f"xp{pair}", tag=f"xp{pair}")
            nc.sync.dma_start(t[:, 0], xp_d[pair, :, 0])
            nc.scalar.dma_start(t[:, 1], xp_d[pair, :, 1])
            XP.append(t)
        AN_sb = xpool.tile([128, 2, 2, 16], FP8, name="anyq", tag="anyq")
        nc.sync.dma_start(AN_sb[:], AN_d[:])

        def load_a_tiles(ci, first):
            klo, kw = CHUNKS[ci]
            row = []
            for q in range(6):
                rr = []
                for pair in range(2):
                    t = apool.tile([128, 2, KW], FP8, name=f"a{q}{pair}",
                                   tag=f"a{q}{pair}")
                    if first:
                        eng = (nc.scalar, nc.sync, nc.gpsimd)[(2 * q + pair) % 3]
                    else:
                        eng = nc.gpsimd if (q + pair) % 2 == 0 else nc.sync
                    eng.dma_start(t[:, :, :kw], A_d[q, pair, :, :, klo:klo + kw])
                    rr.append(t)
                row.append(rr)
            return row

        a_pref = load_a_tiles(0, True)
        ones_bf = consts.tile([NSZ, 1], BF16, name="onesbf", tag="onesbf")
        nc.vector.memset(ones_bf[:], 1.0)

        # ---- non-critical loads, issued on idle queues at startup ----
        xb_sb = []
        for nt in range(NT):
            t = xpool.tile([NSZ, C], BF16, name=f"xb{nt}", tag=f"xb{nt}")
            nc.gpsimd.dma_start(t[:], xb_d[nt * NSZ:(nt + 1) * NSZ, :])
            xb_sb.append(t)
        ones1x64 = consts.tile([1, 64], F32, name="o1x64", tag="o1x64")
        nc.vector.memset(ones1x64[:], 1.0)
        ones1x128 = consts.tile([1, 128], F32, name="o1x128", tag="o1x128")
        nc.vector.memset(ones1x128[:], 1.0)
        onesP64 = consts.tile([64, 1], F32, name="oP64", tag="oP64")
        nc.vector.memset(onesP64[:], 1.0)
        onesP128 = consts.tile([128, 1], F32, name="oP128", tag="oP128")
        nc.vector.memset(onesP128[:], 1.0)
        eps128 = consts.tile([128, 1], F32, name="eps128", tag="eps128")
        nc.vector.memset(eps128[:], EPS)
        W_sb = []
        for i in range(3):
            t = consts.tile([32, 64], F32, name=f"W{i}", tag=f"W{i}")
            nc.gpsimd.dma_start(t[:], W_d[i])
            W_sb.append(t)
        CW_sb = []
        for i in range(4):
            t = consts.tile([64, 64], F32, name=f"CW{i}", tag=f"CW{i}")
            nc.gpsimd.dma_start(t[:], CW_d[i])
            CW_sb.append(t)
        G_sb = []
        for i in range(2):
            t = consts.tile([64, 64], F32, name=f"G{i}", tag=f"G{i}")
            nc.gpsimd.dma_start(t[:], G_d[i])
            G_sb.append(t)
        UV_sb = []
        for i in range(4):
            t = consts.tile([1, 64], F32, name=f"uv{i}", tag=f"uv{i}")
            nc.gpsimd.dma_start(t[:], UV_d[i:i + 1, :])
            UV_sb.append(t)

        # ---- Nyquist bin 2048: xfN[n, t] = sum_c x[n,c] s_t(c) (-1)^{h_t(c)}
        # (s_t pre-scaled by SCALE on host). Hidden under the chunk-0 A DMA.
        with tc.tile_pool(name="psN0", bufs=2, space=PSUM) as psN0, \
             tc.tile_pool(name="psN1", bufs=1, space=PSUM) as psN1, \
             tc.tile_pool(name="nyq", bufs=1) as nyq:
            cpn = nyq.tile([NSZ, 2 * NT], F32, name="cpn", tag="cpn")
            nyx = nyq.tile([NSZ, 3 * NT], F32, name="nyx", tag="nyx")
            for nt in range(NT):
                pn = psN0.tile([NSZ, 3], F32, name="pn", tag="pn")
                for pair in range(2):
                    nc.tensor.matmul(
                        pn[:], XP[pair][:, :, nt * NSZ:(nt + 1) * NSZ],
                        AN_sb[:, pair, :, 0:3],
                        start=(pair == 0), stop=(pair == 1),
                        perf_mode=PM.DoubleRow)
                xn = nyx[:, 3 * nt:3 * nt + 3]
                nc.vector.tensor_copy(xn, pn[:])
                # cp1N = xf0N*xf1N ; cp2N = cp1N*xf2N
                nc.vector.tensor_mul(cpn[:, 2 * nt:2 * nt + 1],
                                     xn[:, 0:1], xn[:, 1:2])
                nc.vector.tensor_mul(cpn[:, 2 * nt + 1:2 * nt + 2],
                                     cpn[:, 2 * nt:2 * nt + 1], xn[:, 2:3])
            ones32 = nyq.tile([NSZ, 1], F32, name="ones32", tag="ones32")
            nc.vector.memset(ones32[:], 1.0)
            pm = psN1.tile([1, 2 * NT], F32, name="pmn", tag="pmn")
            nc.tensor.matmul(pm[:], ones32[:], cpn[:], start=True, stop=True)
            mn = sfin.tile([1, 4], F32, name="mn", tag="mn")
            # m1R[2048], m2R[2048]; imag parts are zero
            nc.vector.tensor_reduce(
                mn[:, 0:1], pm[:].rearrange("p (a b) -> p a b", b=2)[:, :, 0],
                AX.X, ALU.add)
            nc.vector.tensor_reduce(
                mn[:, 2:3], pm[:].rearrange("p (a b) -> p a b", b=2)[:, :, 1],
                AX.X, ALU.add)
            nc.vector.memset(mn[:, 1:2], 0.0)
            nc.vector.memset(mn[:, 3:4], 0.0)
            for qi in (0, 2):
                nc.gpsimd.dma_start(mrow_d[qi][:, 2048:2049], mn[:, qi:qi + 1])


        # ---- first-order term early: rides the startup DMA window ----
        # first = a1 * mean_n x (per channel)
        absf, sgnf = [], []
        with tc.tile_pool(name="psF", bufs=4, space=PSUM) as psF:
            for ct in range(4):
                fp = psF.tile([128, 1], F32, name="fp", tag="fp")
                for nt in range(NT):
                    nc.tensor.matmul(
                        fp[:], xb_sb[nt][:, ct * 128:(ct + 1) * 128],
                        ones_bf[:],
                        start=(nt == 0), stop=(nt == NT - 1))
                av = sfin.tile([128, 1], F32, name=f"absf{ct}", tag=f"absf{ct}")
                nc.scalar.activation(av[:], fp[:], AF.Abs, scale=s1scale)
                sv = sfin.tile([128, 1], F32, name=f"sgnf{ct}", tag=f"sgnf{ct}")
                nc.scalar.activation(sv[:], fp[:], AF.Sign, scale=s1sign)
                absf.append(av)
                sgnf.append(sv)

        with tc.tile_pool(name="xfpool", bufs=2) as xfpool, \
             tc.tile_pool(name="cppool", bufs=2) as cppool, \
             tc.tile_pool(name="tmppool", bufs=1) as tmppool, \
             tc.tile_pool(name="psA", bufs=3, space=PSUM) as psA, \
             tc.tile_pool(name="psM", bufs=2, space=PSUM) as psM:

            pending = []

            def emit_mred(job):
                cpt, klo, kw = job
                last = klo + kw >= 2048
                for qi in range(4):
                    ps = psM.tile([1, KW], F32, name="psm", tag="psm")
                    for j in range(NT):
                        nc.tensor.matmul(
                            ps[:, :kw], ones_bf[:],
                            cpt[qi][:, j, :kw],
                            start=(j == 0), stop=(j == NT - 1))
                    st = stpool.tile([1, KW], F32, name="mstage", tag="mstage")
                    nc.scalar.copy(st[:, :kw], ps[:, :kw])
                    nc.sync.dma_start(
                        mrow_d[qi][:, klo:klo + kw], st[:, :kw])

            def emit_products_half(xf, cp, tA, tB, lo, hi, kw):
                # stage B on position tiles [lo, hi): cp1=xf0*xf1, cp2=cp1*xf2
                sl = (slice(None), slice(lo, hi), slice(0, kw))
                R0, I0, R1, I1, R2, I2 = (t[sl] for t in xf)
                cp1R, cp1I, cp2R, cp2I = (t[sl] for t in cp)
                a, b = tA[sl], tB[sl]
                nc.vector.tensor_mul(a, R0, R1)
                nc.vector.tensor_mul(b, I0, I1)
                nc.vector.tensor_sub(cp1R, a, b)
                nc.vector.tensor_mul(a, R0, I1)
                nc.vector.tensor_mul(b, I0, R1)
                nc.vector.tensor_add(cp1I, a, b)
                nc.vector.tensor_mul(a, cp1R, R2)
                nc.vector.tensor_mul(b, cp1I, I2)
                nc.vector.tensor_sub(cp2R, a, b)
                nc.vector.tensor_mul(a, cp1R, I2)
                nc.vector.tensor_mul(b, cp1I, R2)
                nc.vector.tensor_add(cp2I, a, b)

            for ci in range(NCH):
                klo, kw = CHUNKS[ci]
                a_sb = a_pref if ci == 0 else load_a_tiles(ci, False)

                # stage A: xf_q[n, k] = SCALE * sum_c x[n,c] A_q[c,k]
                # q-pairs share one [NSZ, 2, KW] psum tile -> one evac copy
                xfg = [xfpool.tile([NSZ, NT, 2, KW], BF16, name=f"xfg{g}",
                                   tag=f"xfg{g}") for g in range(3)]
                xf = [xfg[q // 2][:, :, q % 2, :] for q in range(6)]
                cp = [cppool.tile([NSZ, NT, KW], BF16, name=f"cp{i}", tag=f"cp{i}")
                      for i in range(4)]
                tA = tmppool.tile([NSZ, NT, KW], BF16, name="tA", tag="tA")
                tB = tmppool.tile([NSZ, NT, KW], BF16, name="tB", tag="tB")
                for nt in range(NT):
                    for g in range(3):
                        ps = psA.tile([NSZ, 2, KW], F32, name="psa", tag="psa")
                        for qq in range(2):
                            for pair in range(2):
                                nc.tensor.matmul(
                                    ps[:, qq, :kw],
                                    XP[pair][:, :, nt * NSZ:(nt + 1) * NSZ],
                                    a_sb[2 * g + qq][pair][:, :, :kw],
                                    start=(pair == 0), stop=(pair == 1),
                                    perf_mode=PM.DoubleRow)
                        nc.scalar.activation(xfg[g][:, nt, :, :kw], ps[:, :, :kw],
                                             AF.Copy, scale=SCALE)
                    if nt == 4:
                        # first-half products overlap this chunk's stage A
                        emit_products_half(xf, cp, tA, tB, 0, 5, kw)
                emit_products_half(xf, cp, tA, tB, 5, NT, kw)

                pending.append((cp, klo, kw))
                if ci >= 1:
                    emit_mred(pending.pop(0))
            while pending:
                emit_mred(pending.pop(0))

        # ================= final phase =================
        with tc.tile_pool(name="psT", bufs=1, space=PSUM) as psT, \
             tc.tile_pool(name="psY", bufs=1, space=PSUM) as psY, \
             tc.tile_pool(name="psN", bufs=1, space=PSUM) as psN, \
             tc.tile_pool(name="psB", bufs=1, space=PSUM) as psB:

            y_ps = []
            s_t = []
            mmTs = []
            for t in range(2):
                mmT = []
                for q in range(2):  # R, I
                    mt = fin.tile([32, 64], F32, name=f"mmT{t}{q}", tag=f"mmT{t}{q}")
                    nc.sync.dma_start(
                        mt[:],
                        mrow_d[2 * t + q][:, 0:2048]
                        .rearrange("p (a b) -> (p a) b", a=32))
                    mmT.append(mt)
                mmTs.append(mmT)
            m0s = [mmTs[0][0][0:1, 0:1], mmTs[1][0][0:1, 0:1]]
            mNs = [mn[:, 0:1], mn[:, 2:3]]
            TRs, TIs, Tps, crows = [], [], [], []
            for t in range(2):  # stage 1 DFT for both orders first
                mmT = mmTs[t]
                TR = psT.tile([64, 64], F32, name="TR", tag=f"TR{t}")
                nc.tensor.matmul(TR[:], mmT[0][:], W_sb[0][:], start=True, stop=False)
                nc.tensor.matmul(TR[:], mmT[1][:], W_sb[2][:], start=False, stop=True)
                TI = psT.tile([64, 64], F32, name="TI", tag=f"TI{t}")
                nc.tensor.matmul(TI[:], mmT[0][:], W_sb[1][:], start=True, stop=False)
                nc.tensor.matmul(TI[:], mmT[1][:], W_sb[0][:], start=False, stop=True)
                TRs.append(TR)
                TIs.append(TI)
            for t in range(2):  # twiddle + correction row (DVE)
                TR, TI = TRs[t], TIs[t]
                CR, CI = CW_sb[2 * t], CW_sb[2 * t + 1]
                ta = fin.tile([64, 64], F32, name=f"ta{t}", tag=f"ta{t}")
                tb = fin.tile([64, 64], F32, name=f"tb{t}", tag=f"tb{t}")
                TpR = fin.tile([64, 64], F32, name=f"TpR{t}", tag=f"TpR{t}")
                TpI = fin.tile([64, 64], F32, name=f"TpI{t}", tag=f"TpI{t}")
                nc.vector.tensor_mul(ta[:], TR[:], CR[:])
                nc.vector.tensor_mul(tb[:], TI[:], CI[:])
                nc.vector.tensor_sub(TpR[:], ta[:], tb[:])
                nc.vector.tensor_mul(ta[:], TR[:], CI[:])
                nc.vector.tensor_mul(tb[:], TI[:], CR[:])
                nc.vector.tensor_add(TpI[:], ta[:], tb[:])
                crow = fin.tile([1, 64], F32, name=f"crow{t}", tag=f"crow{t}")
                tmpr = fin.tile([1, 64], F32, name=f"tmpr{t}", tag=f"tmpr{t}")
                nc.vector.tensor_scalar_mul(tmpr[:], UV_sb[2 * t + 1][:], mNs[t])
                nc.vector.scalar_tensor_tensor(
                    crow[:], UV_sb[2 * t][:], m0s[t], tmpr[:],
                    op0=ALU.mult, op1=ALU.add)
                Tps.append((TpR, TpI))
                crows.append(crow)
            for t in range(2):  # stage 2 + correction broadcast
                TpR, TpI = Tps[t]
                y = psY.tile([64, 64], F32, name=f"y{t}", tag=f"y{t}")
                nc.tensor.matmul(y[:], G_sb[0][:], TpR[:], start=True, stop=False)
                nc.tensor.matmul(y[:], G_sb[1][:], TpI[:], start=False, stop=False)
                nc.tensor.matmul(y[:], ones1x64[:], crows[t][:], start=False,
                                 stop=True, skip_group_check=True)
                y_ps.append(y)
                st = fin.tile([64, 1], F32, name=f"st{t}", tag=f"st{t}")
                nc.vector.tensor_reduce(st[:], y[:], AX.X, ALU.add,
                                        apply_absolute_value=True)
                s_t.append(st)

            # norm total = sum|y1| + sum|y2| + sum|first| + (|a0| + NPHI*eps)
            tot = psN.tile([1, 1], F32, name="tot", tag="tot")
            nc.tensor.matmul(tot[:], onesP64[:], s_t[0][:], start=True, stop=False,
                             skip_group_check=True)
            nc.tensor.matmul(tot[:], onesP64[:], s_t[1][:], start=False, stop=False,
                             skip_group_check=True)
            for ct in range(4):
                nc.tensor.matmul(tot[:], onesP128[:], absf[ct][:],
                                 start=False, stop=(ct == 3),
                                 skip_group_check=True)
            tot_sb = fin.tile([1, 1], F32, name="tot_sb", tag="tot_sb")
            nc.scalar.activation(tot_sb[:], tot[:], AF.Copy, bias=c0)
            rec = fin.tile([1, 1], F32, name="rec", tag="rec")
            nc.vector.reciprocal(rec[:], tot_sb[:])
            ninv = fin.tile([1, 1], F32, name="ninv", tag="ninv")
            nc.scalar.sqrt(ninv[:], rec[:])
            nv128_ps = psB.tile([128, 1], F32, name="nv128", tag="nv128")
            nc.tensor.matmul(nv128_ps[:], ones1x128[:], ninv[:], start=True, stop=True)
            nv128 = fin.tile([128, 1], F32, name="nv128sb", tag="nv128sb")
            nc.scalar.copy(nv128[:], nv128_ps[:])
            nv64 = nv128[0:64]

            # phi pieces
            ph0 = fin.tile([1, 1], F32, name="ph0", tag="ph0")
            nc.vector.tensor_scalar_mul(ph0[:], ninv[:], zsigned)
            nc.sync.dma_start(phi0_d[:], ph0[:])
            for ct in range(4):
                sqf = fin.tile([128, 1], F32, name=f"sqf{ct}", tag=f"sqf{ct}")
                nc.scalar.activation(sqf[:], absf[ct][:], AF.Sqrt, bias=eps128[:])
                pmf = fin.tile([128, 1], F32, name=f"pmf{ct}", tag=f"pmf{ct}")
                nc.vector.tensor_mul(pmf[:], sqf[:], sgnf[ct][:])
                phf = fin.tile([128, 1], F32, name=f"phf{ct}", tag=f"phf{ct}")
                nc.vector.tensor_scalar_mul(phf[:], pmf[:], nv128[:])
                nc.sync.dma_start(pfirst_d[ct * 128:(ct + 1) * 128, :], phf[:])
            for t in range(2):
                ab = fin.tile([64, 64], F32, name=f"ab{t}", tag=f"ab{t}")
                nc.scalar.activation(ab[:], y_ps[t][:], AF.Abs)
                sq = fin.tile([64, 64], F32, name=f"sq{t}", tag=f"sq{t}")
                nc.scalar.activation(sq[:], ab[:], AF.Sqrt, bias=eps128[:64])
                sg = fin.tile([64, 64], F32, name=f"sg{t}", tag=f"sg{t}")
                nc.scalar.activation(sg[:], y_ps[t][:], AF.Sign)
                pm = fin.tile([64, 64], F32, name=f"pm{t}", tag=f"pm{t}")
                nc.vector.tensor_mul(pm[:], sq[:], sg[:])
                phx = fin.tile([64, 64], F32, name=f"phx{t}", tag=f"phx{t}")
                nc.vector.tensor_scalar_mul(phx[:], pm[:], nv64[:])
                nc.sync.dma_start(pxi_d[t][:], phx[:])

    nc.compile()
    return nc


def _host_prep(x, alpha, h_idx, s_bits):
    """Per-core input maps: fp8 image/DFT layouts + fp32 IFFT constants."""
    x = np.asarray(x, np.float32)
    alpha = np.asarray(alpha, np.float64)
    h_idx = np.asarray(h_idx).astype(np.int64)
    s_bits = np.asarray(s_bits).astype(np.int64)
    signs = (2 * s_bits - 1).astype(np.float64)

    # DFT matrices A_t[c, k] (fp8), packed for DoubleRow:
    # A8[q, pair, p, i, k] = A_q[pair*256 + i*128 + p, k]
    k = np.arange(2048, dtype=np.float64)[:, None]
    Abig = np.empty((6, C, 2048), ml_dtypes.float8_e4m3)
    for t in range(3):
        ang = -2.0 * np.pi * ((k * h_idx[t][None, :]) % D) / D
        Abig[2 * t] = (np.cos(ang) * signs[t][None, :]).T.astype(
            ml_dtypes.float8_e4m3)
        Abig[2 * t + 1] = (np.sin(ang) * signs[t][None, :]).T.astype(
            ml_dtypes.float8_e4m3)
    A8 = np.ascontiguousarray(
        Abig.reshape(6, 2, 2, 128, 2048).transpose(0, 1, 3, 2, 4))
    # AN8[p, pair, i, t] = SCALE * s_t(c) * (-1)^{h_t(c)},  c = pair*256+i*128+p
    Anyq = np.empty((C, 3), np.float64)
    for t in range(3):
        Anyq[:, t] = SCALE * signs[t] * ((-1.0) ** (h_idx[t] % 2))
    AN8 = np.zeros((128, 2, 2, 16), ml_dtypes.float8_e4m3)
    AN8[:, :, :, 0:3] = np.ascontiguousarray(
        Anyq.reshape(2, 2, 128, 3).transpose(2, 0, 1, 3)).astype(
        ml_dtypes.float8_e4m3)

    j0 = np.arange(64, dtype=np.float64)[None, :]
    k2 = np.arange(32, dtype=np.float64)[:, None]
    k1 = np.arange(64, dtype=np.float64)[:, None]
    Wc = np.empty((3, 32, 64), np.float32)
    Wc[0] = np.cos(2 * np.pi * k2 * j0 / 64)
    Wc[1] = np.sin(2 * np.pi * k2 * j0 / 64)
    Wc[2] = -Wc[1]
    Cw = np.empty((4, 64, 64), np.float32)
    uv = np.empty((4, 64), np.float32)
    for t in range(2):
        # undo the SCALE^(t+2) applied on-device to cp_{t+1}
        sig = 2.0 * alpha[2 + t] / (D * N) / SCALE ** (t + 2)
        Cw[2 * t] = sig * np.cos(2 * np.pi * k1 * j0 / D)
        Cw[2 * t + 1] = sig * np.sin(2 * np.pi * k1 * j0 / D)
        uv[2 * t] = -alpha[2 + t] / (D * N) / SCALE ** (t + 2)
        uv[2 * t + 1] = (alpha[2 + t] / (D * N) / SCALE ** (t + 2)
                         * ((-1.0) ** np.arange(64)))
    g = 2 * np.pi * k1 * np.arange(64)[None, :] / 64
    Gc = np.empty((2, 64, 64), np.float32)
    Gc[0] = np.cos(g)
    Gc[1] = -np.sin(g)

    in_maps = []
    xf = x.reshape(B, N, C)
    for b in range(B):
        # xp8[pair, p, i, n] = x[n, pair*256 + i*128 + p]
        xT = xf[b].T.reshape(2, 2, 128, N).transpose(0, 2, 1, 3)
        in_maps.append({
            "xb": xf[b].astype(ml_dtypes.bfloat16),
            "xp8": np.ascontiguousarray(xT).astype(ml_dtypes.float8_e4m3),
            "Abig": A8, "Anyq": AN8, "Wc": Wc, "Cw": Cw, "Gc": Gc, "uv": uv,
        })
    return in_maps, float(alpha[0]), float(alpha[1])


def kernel(x, alpha, h_idx, s_bits, _trace=False, _tmpdir=None):
    in_maps, a0, a1 = _host_prep(x, alpha, h_idx, s_bits)
    key = (round(a0, 12), round(a1, 12))
    if key not in _cache:
        _cache[key] = _build_program(a0, a1)
    nc = _cache[key]
    res = run_bass_kernel_spmd(nc, in_maps, core_ids=list(range(B)),
                               trace=_trace, tmpdir=_tmpdir)
    kernel.last_result = res
    out = np.empty((B, NPHI), np.float32)
    for b in range(B):
        r = res.results[b]
        out[b, 0] = r["phi0"][0, 0]
        out[b, 1:1 + C] = r["pfirst"].reshape(C)
        out[b, 1 + C:1 + C + D] = r["pxi1"].reshape(D)
        out[b, 1 + C + D:] = r["pxi2"].reshape(D)
    return out
